# revision 1
# baseline (speedup 1.0000x reference)
"""Trainium2 Bass kernel for DDGAttention (N=4, L=1024, D=128, H=12, DQK=DV=16).

Sharding: 8 cores = 4 batch x 2 query-halves of 512. Each core runs dense
512x1024 attention for all 12 heads plus the geometric epilogue; the host
shards inputs / gathers outputs (no collectives).

Design notes (per-core):
 - q/k/v projections on the host (tiny GEMMs); device gets kT/qT pre-packed
   into 32-partition strips (head 4g+t at partitions 32t..32t+16 of group
   tensor g) and A' = [v_h | pos_CB | 1] packed per key block.
 - logits computed transposed [j, i] (lhsT = kT strip, rhs = qT strip), so
   E = exp(logits^T) feeds the AV matmul as the STATIONARY operand:
   AV out[i, c] = E_chunk^T @ A' with A' [128, 20] moving. Attention output
   lands directly in [query-partition, feature] layout -- no transposes, no
   big PSUM->SBUF copies -- and the PE streams only 20 cols per AV matmul.
 - ACT (exp over 12*512*1024 logits = ~47us busy) is the bottleneck; the
   schedule keeps it saturated: software-pipelined emission (logits matmuls
   of group n+1/n+2 before AV matmuls of group n) so the strict-FIFO PE
   never blocks the exp chain, and double-buffered [128, 1536] (3-bank)
   logit tiles amortize the per-instruction ACT overhead over 3-half spans
   (the all-ones-mask build has no per-key bias, so exps fuse across key
   blocks; the general build uses [128, 1024] spans with per-block bias).
 - softmax denominator = the ones-column of A'; rel_pos aggregation uses
   alpha @ rel_pos = alpha @ pos_CB - pos_CA * rowsum(alpha); no
   max-subtraction (logits are O(20), fp32 exp is safe).
 - every sqrt is computed as exp(0.5*ln(x)) and the ACT table list is pinned
   to natural_log_exp_and_others, so the kernel performs exactly ONE ~2.7us
   table load (no exp<->sqrt switches); distance/norm squares are batched
   into one [128, IC*24] buffer and sqrt-ed by a single ln+exp pair (a
   second tiny pair covers the last two heads in the tail).
 - geometric epilogue is emitted as head-groups complete: heads 0..7 and
   8..9 run under the remaining attention; only heads 10..11 plus the
   Wo/LayerNorm chain remain after the last exp. DVE ops are batched over
   all 4 query chunks (3-level APs) to cut per-op overhead.
 - fp16 operands on the PE-heavy paths, bf16 for E (needs fp32-range
   exponent), fp32 PSUM accumulation and fp32 residual + LayerNorm.
 - a "trivial" build (mask all-ones, bo=0, gamma=1, beta=0 -- the shipped
   setup_inputs) skips masking/affine ops and uses the fused exp spans; the
   general variant (auto-selected otherwise) keeps per-key-block exp bias.
"""

import numpy as np
import ml_dtypes

import concourse.bass as bass
import concourse.mybir as mybir
from concourse.tile import TileContext
from concourse.masks import make_identity
from concourse import bacc, bass_utils
import concourse.bacc as _bacc_mod
from concourse.hw_specs import get_activation_tables as _orig_act_tables


def _only_ln_exp_tables(arch):
    """Keep only natural_log_exp_and_others (ids preserved): the kernel uses
    exp/ln/copy exclusively, so one ACT table load suffices."""
    tabs = _orig_act_tables(arch)
    return {k: (v if k == "natural_log_exp_and_others" else set())
            for k, v in tabs.items()}


F32 = mybir.dt.float32
BF16 = mybir.dt.bfloat16
F16 = mybir.dt.float16
AF = mybir.ActivationFunctionType
ALU = mybir.AluOpType

N, L, D = 4, 1024, 128
H, DQK, DV = 12, 16, 16
NCORES = 8
JB = 8          # key blocks of 128
IC = 4          # query chunks of 128 (per 512-half)
G = 3           # head groups of 4
EPS_LN = 1e-5
INF = 1e5

# geo parts: (hlo, hhi, d2 col, n2 col) within the per-ic 24-col collector
_PARTS = {0: (0, 8, 0, 8), "1a": (8, 10, 16, 18), "1b": (10, 12, 20, 22)}

_compiled = {}


def _bap(ap, free_ap):
    """AP with replaced free dims (for 0-step broadcast reads)."""
    return bass.AP(tensor=ap.tensor, offset=ap.offset, ap=[ap.ap[0]] + free_ap)


def _build(reps=1, trivial=False):
    nc = bacc.Bacc(trn_type="TRN2")

    # ---- I/O ----------------------------------------------------------
    qtp = nc.dram_tensor("qtp", [128, G * 512], F16, kind="ExternalInput")
    ktp = nc.dram_tensor("ktp", [128, G * L], F16, kind="ExternalInput")
    apkh = nc.dram_tensor("apkh", [128, JB * H * 20], BF16, kind="ExternalInput")
    xq = nc.dram_tensor("xq", [128, IC * 128], F32, kind="ExternalInput")
    pca = nc.dram_tensor("pca", [128, IC * 3], F32, kind="ExternalInput")
    frm = nc.dram_tensor("frm", [128, IC * 9], F32, kind="ExternalInput")
    expb = nc.dram_tensor("expb", [128, JB], F32, kind="ExternalInput")
    mski = nc.dram_tensor("mski", [128, IC], F32, kind="ExternalInput")
    wo01 = nc.dram_tensor("wo01", [256, 128], F16, kind="ExternalInput")
    wo2 = nc.dram_tensor("wo2", [20, 128], F16, kind="ExternalInput")
    bob = nc.dram_tensor("bob", [128, 128], F32, kind="ExternalInput")
    gmb = nc.dram_tensor("gmb", [128, 128], F32, kind="ExternalInput")
    btb = nc.dram_tensor("btb", [128, 128], F32, kind="ExternalInput")
    out = nc.dram_tensor("out", [IC * 128, 128], F32, kind="ExternalOutput")

    with TileContext(nc) as tc:
        with tc.tile_pool(name="sing", bufs=1) as sing, \
             tc.tile_pool(name="epool", bufs=2) as epool, \
             tc.tile_pool(name="ep", bufs=4) as ep, \
             tc.tile_pool(name="pslg", bufs=2 if trivial else 3,
                          space="PSUM") as pslg, \
             tc.tile_pool(name="psav", bufs=2, space="PSUM") as psav:

            # ---- load constants / inputs (critical-path DMAs first) ---
            identb = sing.tile([128, 128], F16)
            make_identity(nc, identb)
            ktp_sb = sing.tile([128, G, L], F16)    # [16d strips, g, j]
            qtp_sb = sing.tile([128, G, 512], F16)  # [16d strips, g, i]
            apk = sing.tile([128, JB, H, 20], BF16)
            kr = ktp[:].rearrange("p (g j) -> p g j", g=G)
            qr = qtp[:].rearrange("p (g i) -> p g i", g=G)
            # two parallel critical queues: everything the first exps need
            nc.sync.dma_start(out=qtp_sb[:, 0, :], in_=qr[:, 0, :])
            nc.scalar.dma_start(out=ktp_sb[:, 0, 0:256], in_=kr[:, 0, 0:256])
            if not trivial:
                expb_sb = sing.tile([128, JB], F32)
                nc.scalar.dma_start(out=expb_sb, in_=expb[:])
            nc.sync.dma_start(out=ktp_sb[:, 0, 256:1024], in_=kr[:, 0, 256:1024])
            nc.sync.dma_start(out=apk, in_=apkh[:].rearrange(
                "p (b h c) -> p b h c", b=JB, h=H))
            # bulk queue (gpsimd/SWDGE): later groups + epilogue inputs
            for _g in (1, 2):
                nc.gpsimd.dma_start(out=ktp_sb[:, _g, :], in_=kr[:, _g, :])
                nc.gpsimd.dma_start(out=qtp_sb[:, _g, :], in_=qr[:, _g, :])
            xq_sb = sing.tile([128, IC, 128], F32)
            nc.gpsimd.dma_start(out=xq_sb,
                                in_=xq[:].rearrange("p (b d) -> p b d", b=IC))
            pca_sb = sing.tile([128, IC, 3], F32)
            nc.gpsimd.dma_start(out=pca_sb,
                                in_=pca[:].rearrange("p (b c) -> p b c", b=IC))
            frm_sb = sing.tile([128, IC, 9], F32)
            nc.gpsimd.dma_start(out=frm_sb,
                                in_=frm[:].rearrange("p (b c) -> p b c", b=IC))
            mski_sb = sing.tile([128, IC], F32)
            nc.gpsimd.dma_start(out=mski_sb, in_=mski[:])
            wo0_sb = sing.tile([128, 128], F16)
            nc.gpsimd.dma_start(out=wo0_sb, in_=wo01[0:128, :])
            wo1_sb = sing.tile([128, 128], F16)
            nc.gpsimd.dma_start(out=wo1_sb, in_=wo01[128:256, :])
            wo2_sb = sing.tile([20, 128], F16)
            nc.gpsimd.dma_start(out=wo2_sb, in_=wo2[:])
            bob_sb = sing.tile([128, 128], F32)
            nc.gpsimd.dma_start(out=bob_sb, in_=bob[:])
            gmb_sb = sing.tile([128, 128], F32)
            nc.gpsimd.dma_start(out=gmb_sb, in_=gmb[:])
            btb_sb = sing.tile([128, 128], F32)
            nc.gpsimd.dma_start(out=btb_sb, in_=btb[:])
            eps_sb = sing.tile([128, 1], F32)
            nc.vector.memset(eps_sb, EPS_LN)
            tiny_sb = sing.tile([128, 1], F32)
            nc.vector.memset(tiny_sb, 1e-30)
            warm = sing.tile([128, 1], F32)
            nc.scalar.activation(out=warm, in_=eps_sb, func=AF.Exp)
            # PE warm-up during the input-DMA wait: dummy matmuls on a
            # DVE-zeroed tile (ready at ~0.1us -- independent of DMAs and of
            # the gpsimd identity build) release the HAM clock throttle so
            # the first real matmuls run at full rate. Sized to finish before
            # the first inputs land, so they never delay real work.
            wz = sing.tile([128, 128], F16)
            nc.vector.memset(wz, 0.0)
            pewu = pslg.tile([128, 128], F32, tag="lg", name="pewu")
            for _ in range(20):
                nc.tensor.matmul(pewu, wz, wz, start=True, stop=True)

            # (reps>1 only for replication-slope timing)
            def _one_pass():
              if trivial:
                  xbo = xq_sb
              else:
                  xbo = sing.tile([128, IC, 128], F32)
                  for ic in range(IC):
                      nc.vector.scalar_tensor_tensor(
                          out=xbo[:, ic, :], in0=bob_sb,
                          scalar=mski_sb[:, ic:ic + 1],
                          in1=xq_sb[:, ic, :], op0=ALU.mult, op1=ALU.add)

              # attention output, [i-part, ic, h*20+c] (c: 16 v | 3 pos | den)
              ft_all = sing.tile([128, IC, H * 20], F32)
              # distance/norm squares collector (see _PARTS)
              dall = sing.tile([128, IC, 24], F32)
              sqall = sing.tile([128, IC, 24], F32)    # sqrt(dall)
              rsqall = sing.tile([128, IC, 24], F32)   # rsqrt(dall)
              # feat staging [i, ic, c] (f16 for the Wo transposes)
              fa0_a = sing.tile([128, IC, 128], F16)
              fa1_a = sing.tile([128, IC, 128], F16)
              fa2_a = sing.tile([128, IC, 32], F16)
              nc.vector.memset(fa2_a[:, :, 20:32], 0.0)

              msk = [mski_sb[:, ic:ic + 1] for ic in range(IC)]
              defer = {}

              def _emit_geo(part):
                  hlo, hhi, dc, nc_ = _PARTS[part]
                  nh = hhi - hlo
                  f4 = ft_all[:, :, hlo * 20:hhi * 20].rearrange(
                      "p b (h c) -> p b h c", c=20)
                  r = ep.tile([128, IC, nh], F32, tag="s12b", name="rden")
                  nc.vector.reciprocal(
                      r, f4[:, :, :, 19:20].rearrange("p b h o -> p b (h o)"))
                  if not trivial:
                      r2 = ep.tile([128, IC, nh], F32, tag="s12c", name="rm")
                      nc.vector.tensor_mul(
                          r2, r, _bap(mski_sb[:], [[1, IC], [0, nh]]))
                      r = r2
                  if part == 0:
                      node_dst = fa0_a
                  else:
                      nb = (hlo - 8) * 16
                      node_dst = fa1_a[:, :, nb:nb + nh * 16]
                  nc.vector.tensor_mul(
                      node_dst.rearrange("p b (h c) -> p b h c", c=16),
                      f4[:, :, :, 0:16], _bap(r, [[nh, IC], [1, nh], [0, 16]]))
                  if trivial:
                      pcam = pca_sb
                  else:
                      pcam = ep.tile([128, IC, 3], F32, tag="s3", name="pcam")
                      nc.vector.tensor_mul(
                          pcam, pca_sb, _bap(mski_sb[:], [[1, IC], [0, 3]]))
                  pm = ep.tile([128, IC, nh, 3], F32, tag="s36f", name="pm")
                  nc.vector.tensor_mul(pm, f4[:, :, :, 16:19],
                                       _bap(r, [[nh, IC], [1, nh], [0, 3]]))
                  apb = ep.tile([128, IC, nh, 3], F32, tag="s36", name="apb")
                  nc.vector.tensor_sub(apb, pm,
                                       _bap(pcam[:], [[3, IC], [0, nh], [1, 3]]))
                  sq = ep.tile([128, IC, nh, 3], F32, tag="s36b", name="sq")
                  nc.vector.tensor_mul(sq, apb, apb)
                  nc.vector.reduce_sum(out=dall[:, :, dc:dc + nh], in_=sq,
                                       axis=mybir.AxisListType.X)
                  fp = ep.tile([128, IC, nh * 3], F32, tag=f"fp{part}",
                               name="fp")
                  for ic in range(IC):
                      prod = ep.tile([128, nh, 3, 3], F32, tag="s108",
                                     name="prod", bufs=8)
                      nc.vector.tensor_mul(
                          prod,
                          _bap(apb[:, ic, :, :], [[3, nh], [0, 3], [1, 3]]),
                          _bap(frm_sb[:, ic, :], [[0, nh], [3, 3], [1, 3]]))
                      nc.vector.reduce_sum(
                          out=fp[:, ic, :].rearrange("p (x a) -> p x a", a=3),
                          in_=prod.rearrange("p h a b -> p (h a) b"),
                          axis=mybir.AxisListType.X)
                  po = 64 + hlo * 3
                  nc.vector.tensor_copy(fa1_a[:, :, po:po + nh * 3], fp)
                  fsq = ep.tile([128, IC, nh * 3], F32, tag="s36d", name="fsq")
                  nc.vector.tensor_mul(fsq, fp, fp)
                  nc.vector.reduce_sum(
                      out=dall[:, :, nc_:nc_ + nh],
                      in_=fsq.rearrange("p b (x a) -> p b x a", a=3),
                      axis=mybir.AxisListType.X)
                  defer[part] = fp

              def _emit_sqrt(c0, c1):
                  """sqrt + rsqrt of dall[:,:,c0:c1] via one ln + two exps."""
                  lnd = ep.tile([128, IC, c1 - c0], F32, tag="lnd", name="lnd")
                  nc.scalar.activation(out=lnd, in_=dall[:, :, c0:c1],
                                       func=AF.Ln, bias=tiny_sb, scale=1.0)
                  nc.scalar.activation(out=sqall[:, :, c0:c1], in_=lnd,
                                       func=AF.Exp, scale=0.5)
                  nc.scalar.activation(out=rsqall[:, :, c0:c1], in_=lnd,
                                       func=AF.Exp, scale=-0.5)

              def _emit_geo_tail(part):
                  hlo, hhi, dc, nc_ = _PARTS[part]
                  nh = hhi - hlo
                  fp = defer[part]
                  nc.vector.tensor_copy(fa1_a[:, :, 100 + hlo:100 + hhi],
                                        sqall[:, :, dc:dc + nh])
                  rn = rsqall[:, :, nc_:nc_ + 1]
                  dire = ep.tile([128, IC, nh * 3], F32, tag="s36e",
                                 name="dire")
                  nc.vector.tensor_mul(
                      dire.rearrange("p b (h a) -> p b h a", a=3),
                      fp.rearrange("p b (h a) -> p b h a", a=3),
                      _bap(rn, [[24, IC], [1, nh], [0, 3]]))
                  # dir cols 112+3*hlo .. 112+3*hhi, crossing into fa2 at 128
                  lo = 112 + 3 * hlo
                  hi = 112 + 3 * hhi
                  if lo < 128 and hi > 128:
                      nc.vector.tensor_copy(fa1_a[:, :, lo:128],
                                            dire[:, :, 0:128 - lo])
                      nc.vector.tensor_copy(fa2_a[:, :, 0:hi - 128],
                                            dire[:, :, 128 - lo:])
                  elif hi <= 128:
                      nc.vector.tensor_copy(fa1_a[:, :, lo:hi], dire)
                  else:
                      nc.vector.tensor_copy(fa2_a[:, :, lo - 128:hi - 128],
                                            dire)

              avt = {}

              def _emit_ft(g, hlf):
                  av = avt.pop((g, hlf))
                  co = (4 * g + 2 * hlf) * 20
                  nc.vector.tensor_copy(
                      ft_all[:, :, co:co + 40],
                      av[:].rearrange("p (b c) -> p b c", b=IC))

              # a "half" = one head x 512 queries x one key block: the unit
              # of both logits matmuls (N=512) and AV consumption
              halves = [(g, hlf, jb, t2)
                        for g in range(G) for hlf in range(2)
                        for jb in range(JB) for t2 in range(2)]
              # trivial: 3 halves per exp ([128,1536] = 3-bank tile, true
              # double buffering); general: 2 (uniform per-key-block bias)
              gw = 3 if trivial else 2
              groups = [halves[i:i + gw] for i in range(0, len(halves), gw)]

              backlog = []

              def _sqrt_a():
                  _emit_sqrt(0, 20)
                  _emit_geo_tail(0)
                  _emit_geo_tail("1a")
                  defer["sqA_done"] = True

              def _flush(idxs, e):
                  for q, (g, hlf, jb, t2) in enumerate(idxs):
                      first = False
                      if (g, hlf) not in avt:
                          avt[(g, hlf)] = psav.tile(
                              [128, IC * 40], F32, tag="av", name=f"av{g}{hlf}")
                          first = True
                      av = avt[(g, hlf)]
                      h = 4 * g + 2 * hlf + t2
                      for ic in range(IC):
                          nc.tensor.matmul(
                              av[:, ic * 40 + t2 * 20: ic * 40 + t2 * 20 + 20],
                              e[:, q * 512 + ic * 128: q * 512 + (ic + 1) * 128],
                              apk[:, jb, h, :],
                              start=(first and ic == 0),
                              stop=(jb == JB - 1),
                              skip_group_check=True)
                      if jb == JB - 1 and t2 == 1:
                          _emit_ft(g, hlf)
                          if (g, hlf) == (1, 1):
                              backlog.append(lambda: _emit_geo(0))
                          elif (g, hlf) == (2, 0):
                              backlog.append(lambda: _emit_geo("1a"))
                              backlog.append(_sqrt_a)

              pend = []
              for idxs in groups:
                  wide = len(idxs) * 512
                  lg = pslg.tile([128, gw * 512], F32, tag="lg", name="lg")
                  for q, (g, hlf, jb, t2) in enumerate(idxs):
                      t = 2 * hlf + t2
                      nc.tensor.matmul(
                          lg[:, q * 512:(q + 1) * 512],
                          ktp_sb[32 * t:32 * t + 16, g,
                                 jb * 128:(jb + 1) * 128],
                          qtp_sb[32 * t:32 * t + 16, g, :],
                          start=True, stop=True,
                          tile_position=(32 * t, 0))
                  e = epool.tile([128, wide], BF16, tag="E", name="e", bufs=3)
                  if trivial:
                      nc.scalar.activation(out=e, in_=lg[:, 0:wide],
                                           func=AF.Exp, scale=1.0)
                  else:
                      jb = idxs[0][2]
                      nc.scalar.activation(out=e, in_=lg[:, 0:wide],
                                           func=AF.Exp,
                                           bias=expb_sb[:, jb:jb + 1],
                                           scale=1.0)
                  # deferred geo emission: one batch per group, a group late,
                  # so its DVE inputs are long since ready and the strict-FIFO
                  # ACT (for the sqrt batch) never stalls the exp chain
                  if backlog:
                      backlog.pop(0)()
                  pend.append((idxs, e))
                  if len(pend) > 2:
                      _flush(*pend.pop(0))
              while pend:
                  _flush(*pend.pop(0))
              while backlog:
                  backlog.pop(0)()
              if "sqA_done" not in defer:
                  _sqrt_a()

              _emit_geo("1b")
              _emit_sqrt(20, 24)
              _emit_geo_tail("1b")

              # feat_all^T via transposes, then @ Wo ; residual + LN
              for ic in range(IC):
                  wo_ps = psav.tile([128, 128], F32, tag="av", name="wops")
                  fas = [(fa0_a[:, ic, :], 128), (fa1_a[:, ic, :], 128),
                         (fa2_a[:, ic, :], 32)]
                  tp = pslg.tile([128, 384], F16, tag="lg", name="tpa")
                  for cc, (fax, kk) in enumerate(fas):
                      nc.tensor.transpose(tp[0:kk, cc * 128:cc * 128 + 128],
                                          fax, identb)
                  fxt = ep.tile([128, 384], F16, tag="fxt", name="fxt")
                  nc.scalar.copy(fxt[:, 0:256], tp[:, 0:256])
                  nc.vector.tensor_copy(fxt[0:32, 256:384], tp[0:32, 256:384])
                  for cc, kk in enumerate((128, 128, 20)):
                      rhs = (wo0_sb, wo1_sb, wo2_sb)[cc]
                      nc.tensor.matmul(wo_ps[:, 0:128],
                                       fxt[0:kk, cc * 128:cc * 128 + 128], rhs,
                                       start=(cc == 0), stop=(cc == 2))
                  y = ep.tile([128, 128], F32, tag="y", name="y")
                  if trivial:
                      nc.vector.tensor_add(y, wo_ps[:, 0:128], xbo[:, ic, :])
                  else:
                      nc.vector.scalar_tensor_tensor(
                          out=y, in0=wo_ps[:, 0:128], scalar=msk[ic],
                          in1=xbo[:, ic, :], op0=ALU.mult, op1=ALU.add)
                  st6 = ep.tile([128, 6], F32, tag="st6", name="st6")
                  nc.vector.bn_stats(out=st6, in_=y)
                  mv = ep.tile([128, 2], F32, tag="mv", name="mv")
                  nc.vector.bn_aggr(out=mv, in_=st6)
                  # rstd = exp(-0.5 * ln(var + eps))
                  lnv = ep.tile([128, 1], F32, tag="lnv", name="lnv")
                  nc.scalar.activation(out=lnv, in_=mv[:, 1:2], func=AF.Ln,
                                       bias=eps_sb, scale=1.0)
                  rstd = ep.tile([128, 1], F32, tag="rstd", name="rstd")
                  nc.scalar.activation(out=rstd, in_=lnv, func=AF.Exp,
                                       scale=-0.5)
                  xc = ep.tile([128, 128], F32, tag="xc", name="xc")
                  nc.vector.tensor_scalar(out=xc, in0=y, scalar1=mv[:, 0:1],
                                          scalar2=rstd, op0=ALU.subtract,
                                          op1=ALU.mult)
                  if trivial:
                      o1 = xc
                  else:
                      o1 = ep.tile([128, 128], F32, tag="o1", name="o1")
                      nc.vector.tensor_mul(o1, xc, gmb_sb)
                      nc.vector.tensor_add(o1, o1, btb_sb)
                  eng = (nc.sync, nc.scalar, nc.gpsimd, nc.sync)[ic]
                  eng.dma_start(
                      out=out[:].rearrange("(c p) d -> c p d", p=128)[ic], in_=o1)

            for _rep in range(reps):
                _one_pass()

    # force the single ln+exp table set (ids preserved; see helper above)
    _bacc_mod.get_activation_tables = _only_ln_exp_tables
    try:
        nc.compile()
    finally:
        _bacc_mod.get_activation_tables = _orig_act_tables
    return nc


def _pm(a, nb):
    """[nb*128, F] -> partition-major [128, nb*F]."""
    f = a.shape[-1]
    return np.ascontiguousarray(
        a.reshape(nb, 128, f).transpose(1, 0, 2).reshape(128, nb * f))


def kernel(x, pos_CA, pos_CB, frame, mask, Wq, Wk, Wv, Wo, bo, gamma, beta):
    x = np.asarray(x, np.float32)
    pos_CA = np.asarray(pos_CA, np.float32)
    pos_CB = np.asarray(pos_CB, np.float32)
    frame = np.asarray(frame, np.float32)
    maskf = np.asarray(mask).astype(np.float32)
    Wq = np.asarray(Wq, np.float32)
    Wk = np.asarray(Wk, np.float32)
    Wv = np.asarray(Wv, np.float32)
    Wo = np.asarray(Wo, np.float32)
    bo = np.asarray(bo, np.float32)
    gamma = np.asarray(gamma, np.float32)
    beta = np.asarray(beta, np.float32)

    trivial = bool(
        maskf.all()
        and not bo.any()
        and (gamma == 1.0).all()
        and not beta.any()
    )
    key = ("nc", trivial)
    if key not in _compiled:
        _compiled[key] = _build(trivial=trivial)
        _compiled["nc"] = _compiled[key]
    nc = _compiled[key]
    _compiled["nc"] = nc

    wo01 = np.ascontiguousarray(np.vstack([Wo[0:256, :],]))
    wo2 = np.ascontiguousarray(Wo[256:276, :])
    bob = np.ascontiguousarray(np.tile(bo[None, :], (128, 1)))
    gmb = np.ascontiguousarray(np.tile(gamma[None, :], (128, 1)))
    btb = np.ascontiguousarray(np.tile(beta[None, :], (128, 1)))

    in_maps = []
    for c in range(NCORES):
        n, hf = c // 2, c % 2
        xn = x[n]
        sl = slice(hf * 512, (hf + 1) * 512)
        q = xn[sl] @ Wq                       # [512, 192]
        k = xn @ Wk                           # [1024, 192]
        v = xn @ Wv                           # [1024, 192]
        qtp_h = np.zeros((128, G, 512), np.float16)
        ktp_h = np.zeros((128, G, 1024), np.float16)
        for g in range(G):
            for t in range(4):
                h = 4 * g + t
                qtp_h[32 * t:32 * t + 16, g, :] = q[:, h * 16:(h + 1) * 16].T
                ktp_h[32 * t:32 * t + 16, g, :] = k[:, h * 16:(h + 1) * 16].T
        apk_h = np.ones((128, JB, H, 20), ml_dtypes.bfloat16)
        vr = v.reshape(JB, 128, H, 16).transpose(1, 0, 2, 3)
        apk_h[:, :, :, 0:16] = vr.astype(ml_dtypes.bfloat16)
        apk_h[:, :, :, 16:19] = pos_CB[n].reshape(JB, 128, 1, 3).transpose(
            1, 0, 2, 3).astype(ml_dtypes.bfloat16)
        in_maps.append({
            "qtp": qtp_h.reshape(128, G * 512),
            "ktp": ktp_h.reshape(128, G * 1024),
            "apkh": np.ascontiguousarray(apk_h.reshape(128, JB * H * 20)),
            "xq": _pm(xn[sl], 4),
            "pca": _pm(pos_CA[n, sl], 4),
            "frm": _pm(frame[n, sl].reshape(512, 9), 4),
            "expb": np.ascontiguousarray(
                (-INF * (1.0 - maskf[n])).reshape(8, 128).T),
            "mski": np.ascontiguousarray(maskf[n, sl].reshape(4, 128).T),
            "wo01": wo01.astype(np.float16),
            "wo2": wo2.astype(np.float16),
            "bob": bob, "gmb": gmb, "btb": btb,
        })

    res = bass_utils.run_bass_kernel_spmd(nc, in_maps, core_ids=list(range(NCORES)))
    full = np.empty((N, L, D), np.float32)
    for c in range(NCORES):
        n, hf = c // 2, c % 2
        full[n, hf * 512:(hf + 1) * 512, :] = res.results[c]["out"]
    return full



# revision 62
# speedup vs baseline: 10.6637x; 10.6637x over previous
"""Trainium2 Bass kernel for DDGAttention (N=4, L=1024, D=128, H=12, DQK=DV=16).

Sharding: 8 cores = 4 batch x 2 query-halves of 512. Each core runs dense
512x1024 attention for all 12 heads plus the geometric epilogue; the host
shards inputs / gathers outputs (no collectives).

Design notes (per-core):
 - q/k/v projections on the host (tiny GEMMs); device gets kT/qT pre-packed
   into 32-partition strips (head 4g+t at partitions 32t..32t+16 of group
   tensor g) and A' = [v_h | pos_CB | 1] packed per key block.
 - logits computed transposed [j, i] (lhsT = kT strip, rhs = qT strip), so
   E = exp(logits^T) feeds the AV matmul as the STATIONARY operand:
   AV out[i, c] = E_chunk^T @ A' with A' [128, 20] moving. Attention output
   lands directly in [query-partition, feature] layout -- no transposes, no
   big PSUM->SBUF copies -- and the PE streams only 20 cols per AV matmul.
 - ACT (exp over 12*512*1024 logits = ~47us busy) is the bottleneck; the
   schedule keeps it saturated: software-pipelined emission (logits matmuls
   of group n+1/n+2 before AV matmuls of group n) so the strict-FIFO PE
   never blocks the exp chain, and double-buffered [128, 1536] (3-bank)
   logit tiles amortize the per-instruction ACT overhead over 3-half spans
   (the all-ones-mask build has no per-key bias, so exps fuse across key
   blocks; the general build uses [128, 1024] spans with per-block bias).
 - softmax denominator = the ones-column of A'; rel_pos aggregation uses
   alpha @ rel_pos = alpha @ pos_CB - pos_CA * rowsum(alpha); no
   max-subtraction (logits are O(20), fp32 exp is safe).
 - every sqrt is computed as exp(0.5*ln(x)) and the ACT table list is pinned
   to natural_log_exp_and_others, so the kernel performs exactly ONE ~2.7us
   table load (no exp<->sqrt switches); distance/norm squares are batched
   into one [128, IC*24] buffer and sqrt-ed by a single ln+exp pair (a
   second tiny pair covers the last two heads in the tail).
 - geometric epilogue is emitted as head-groups complete: heads 0..7 and
   8..9 run under the remaining attention; only heads 10..11 plus the
   Wo/LayerNorm chain remain after the last exp. DVE ops are batched over
   all 4 query chunks (3-level APs) to cut per-op overhead.
 - fp16 operands on the PE-heavy paths, bf16 for E (needs fp32-range
   exponent), fp32 PSUM accumulation and fp32 residual + LayerNorm.
 - a "trivial" build (mask all-ones, bo=0, gamma=1, beta=0 -- the shipped
   setup_inputs) skips masking/affine ops and uses the fused exp spans; the
   general variant (auto-selected otherwise) keeps per-key-block exp bias.
"""

import numpy as np
import ml_dtypes

import concourse.bass as bass
import concourse.mybir as mybir
from concourse.tile import TileContext
from concourse.masks import make_identity
from concourse import bacc, bass_utils
import concourse.bacc as _bacc_mod
from concourse.hw_specs import get_activation_tables as _orig_act_tables


def _only_ln_exp_tables(arch):
    """Keep only natural_log_exp_and_others (ids preserved): the kernel uses
    exp/ln/copy exclusively, so one ACT table load suffices."""
    tabs = _orig_act_tables(arch)
    return {k: (v if k == "natural_log_exp_and_others" else set())
            for k, v in tabs.items()}


F32 = mybir.dt.float32
BF16 = mybir.dt.bfloat16
F16 = mybir.dt.float16
AF = mybir.ActivationFunctionType
ALU = mybir.AluOpType

N, L, D = 4, 1024, 128
H, DQK, DV = 12, 16, 16
NCORES = 8
JB = 8          # key blocks of 128
IC = 4          # query chunks of 128 (per 512-half)
G = 3           # head groups of 4
EPS_LN = 1e-5
INF = 1e5

# geo parts: (hlo, hhi, d2 col, n2 col) within the per-ic 24-col collector
_PARTS = {0: (0, 8, 0, 8), "1a": (8, 10, 16, 18), "1b": (10, 12, 20, 22)}

_compiled = {}


def _bap(ap, free_ap):
    """AP with replaced free dims (for 0-step broadcast reads)."""
    return bass.AP(tensor=ap.tensor, offset=ap.offset, ap=[ap.ap[0]] + free_ap)


def _build(reps=1, trivial=False):
    nc = bacc.Bacc(trn_type="TRN2")

    # ---- I/O ----------------------------------------------------------
    qtp = nc.dram_tensor("qtp", [128, G * 512], F16, kind="ExternalInput")
    ktp = nc.dram_tensor("ktp", [128, G * L], F16, kind="ExternalInput")
    apkh = nc.dram_tensor("apkh", [128, JB * H * 20], BF16, kind="ExternalInput")
    xq = nc.dram_tensor("xq", [128, IC * 128], F32, kind="ExternalInput")
    pca = nc.dram_tensor("pca", [128, IC * 3], F32, kind="ExternalInput")
    frm = nc.dram_tensor("frm", [128, IC * 9], F32, kind="ExternalInput")
    expb = nc.dram_tensor("expb", [128, JB], F32, kind="ExternalInput")
    mski = nc.dram_tensor("mski", [128, IC], F32, kind="ExternalInput")
    wo01 = nc.dram_tensor("wo01", [256, 128], F16, kind="ExternalInput")
    wo2 = nc.dram_tensor("wo2", [20, 128], F16, kind="ExternalInput")
    bob = nc.dram_tensor("bob", [128, 128], F32, kind="ExternalInput")
    gmb = nc.dram_tensor("gmb", [128, 128], F32, kind="ExternalInput")
    btb = nc.dram_tensor("btb", [128, 128], F32, kind="ExternalInput")
    out = nc.dram_tensor("out", [IC * 128, 128], F32, kind="ExternalOutput")

    with TileContext(nc) as tc:
        with tc.tile_pool(name="sing", bufs=1) as sing, \
             tc.tile_pool(name="epool", bufs=2) as epool, \
             tc.tile_pool(name="ep", bufs=4) as ep, \
             tc.tile_pool(name="pslg", bufs=2 if trivial else 3,
                          space="PSUM") as pslg, \
             tc.tile_pool(name="psav", bufs=2, space="PSUM") as psav:

            # ---- load constants / inputs (critical-path DMAs first) ---
            identb = sing.tile([128, 128], F16)
            make_identity(nc, identb)
            ktp_sb = sing.tile([128, G, L], F16)    # [16d strips, g, j]
            qtp_sb = sing.tile([128, G, 512], F16)  # [16d strips, g, i]
            apk = sing.tile([128, JB, H, 20], BF16)
            kr = ktp[:].rearrange("p (g j) -> p g j", g=G)
            qr = qtp[:].rearrange("p (g i) -> p g i", g=G)
            # two parallel critical queues: everything the first exps need
            nc.sync.dma_start(out=qtp_sb[:, 0, :], in_=qr[:, 0, :])
            nc.scalar.dma_start(out=ktp_sb[:, 0, 0:256], in_=kr[:, 0, 0:256])
            if not trivial:
                expb_sb = sing.tile([128, JB], F32)
                nc.scalar.dma_start(out=expb_sb, in_=expb[:])
            apkr = apkh[:].rearrange("p (b h c) -> p b h c", b=JB, h=H)
            nc.scalar.dma_start(out=apk[:, 0:3], in_=apkr[:, 0:3])
            nc.sync.dma_start(out=ktp_sb[:, 0, 256:768], in_=kr[:, 0, 256:768])
            nc.sync.dma_start(out=ktp_sb[:, 0, 768:1024], in_=kr[:, 0, 768:1024])
            nc.sync.dma_start(out=apk[:, 3:8], in_=apkr[:, 3:8])
            # bulk queue (gpsimd/SWDGE): later groups + epilogue inputs
            for _g in (1, 2):
                nc.gpsimd.dma_start(out=ktp_sb[:, _g, :], in_=kr[:, _g, :])
                nc.gpsimd.dma_start(out=qtp_sb[:, _g, :], in_=qr[:, _g, :])
            xq_sb = sing.tile([128, IC, 128], F32)
            nc.gpsimd.dma_start(out=xq_sb,
                                in_=xq[:].rearrange("p (b d) -> p b d", b=IC))
            pca_sb = sing.tile([128, IC, 3], F32)
            nc.gpsimd.dma_start(out=pca_sb,
                                in_=pca[:].rearrange("p (b c) -> p b c", b=IC))
            frm_sb = sing.tile([128, IC, 9], F32)
            nc.gpsimd.dma_start(out=frm_sb,
                                in_=frm[:].rearrange("p (b c) -> p b c", b=IC))
            mski_sb = sing.tile([128, IC], F32)
            nc.gpsimd.dma_start(out=mski_sb, in_=mski[:])
            wo0_sb = sing.tile([128, 128], F16)
            nc.gpsimd.dma_start(out=wo0_sb, in_=wo01[0:128, :])
            wo1_sb = sing.tile([128, 128], F16)
            nc.gpsimd.dma_start(out=wo1_sb, in_=wo01[128:256, :])
            wo2_sb = sing.tile([20, 128], F16)
            nc.gpsimd.dma_start(out=wo2_sb, in_=wo2[:])
            bob_sb = sing.tile([128, 128], F32)
            nc.gpsimd.dma_start(out=bob_sb, in_=bob[:])
            gmb_sb = sing.tile([128, 128], F32)
            nc.gpsimd.dma_start(out=gmb_sb, in_=gmb[:])
            btb_sb = sing.tile([128, 128], F32)
            nc.gpsimd.dma_start(out=btb_sb, in_=btb[:])
            eps_sb = sing.tile([128, 1], F32)
            nc.vector.memset(eps_sb, EPS_LN)
            tiny_sb = sing.tile([128, 1], F32)
            nc.vector.memset(tiny_sb, 1e-30)
            warm = sing.tile([128, 1], F32)
            nc.scalar.activation(out=warm, in_=eps_sb, func=AF.Exp)
            # PE warm-up during the input-DMA wait: dummy matmuls on a
            # DVE-zeroed tile (ready at ~0.1us -- independent of DMAs and of
            # the gpsimd identity build) release the HAM clock throttle so
            # the first real matmuls run at full rate. Sized to finish before
            # the first inputs land, so they never delay real work.
            wz = sing.tile([128, 128], F16)
            nc.vector.memset(wz, 0.0)
            pewu = pslg.tile([128, 128], F32, tag="lg", name="pewu")
            for _ in range(14):
                nc.tensor.matmul(pewu, wz, wz, start=True, stop=True)

            # (reps>1 only for replication-slope timing)
            def _one_pass():
              if trivial:
                  xbo = xq_sb
              else:
                  xbo = sing.tile([128, IC, 128], F32)
                  for ic in range(IC):
                      nc.vector.scalar_tensor_tensor(
                          out=xbo[:, ic, :], in0=bob_sb,
                          scalar=mski_sb[:, ic:ic + 1],
                          in1=xq_sb[:, ic, :], op0=ALU.mult, op1=ALU.add)

              # attention output, [i-part, ic, h*20+c] (c: 16 v | 3 pos | den)
              ft_all = sing.tile([128, IC, H * 20], F32)
              # distance/norm squares collector (see _PARTS)
              dall = sing.tile([128, IC, 24], F32)
              sqall = sing.tile([128, IC, 24], F32)    # sqrt(dall)
              rsqall = sing.tile([128, IC, 24], F32)   # rsqrt(dall)
              # feat staging [i, ic, c] (f16 for the Wo transposes)
              fa0_a = sing.tile([128, IC, 128], F16)
              fa1_a = sing.tile([128, IC, 128], F16)
              fa2_a = sing.tile([128, IC, 32], F16)
              nc.vector.memset(fa2_a[:, :, 20:32], 0.0)

              msk = [mski_sb[:, ic:ic + 1] for ic in range(IC)]
              defer = {}

              def _emit_geo(part):
                  hlo, hhi, dc, nc_ = _PARTS[part]
                  nh = hhi - hlo
                  f4 = ft_all[:, :, hlo * 20:hhi * 20].rearrange(
                      "p b (h c) -> p b h c", c=20)
                  r = ep.tile([128, IC, nh], F32, tag="s12b", name="rden")
                  nc.vector.reciprocal(
                      r, f4[:, :, :, 19:20].rearrange("p b h o -> p b (h o)"))
                  if not trivial:
                      r2 = ep.tile([128, IC, nh], F32, tag="s12c", name="rm")
                      nc.vector.tensor_mul(
                          r2, r, _bap(mski_sb[:], [[1, IC], [0, nh]]))
                      r = r2
                  if part == 0:
                      node_dst = fa0_a
                  else:
                      nb = (hlo - 8) * 16
                      node_dst = fa1_a[:, :, nb:nb + nh * 16]
                  nc.vector.tensor_mul(
                      node_dst.rearrange("p b (h c) -> p b h c", c=16),
                      f4[:, :, :, 0:16], _bap(r, [[nh, IC], [1, nh], [0, 16]]))
                  if trivial:
                      pcam = pca_sb
                  else:
                      pcam = ep.tile([128, IC, 3], F32, tag="s3", name="pcam")
                      nc.vector.tensor_mul(
                          pcam, pca_sb, _bap(mski_sb[:], [[1, IC], [0, 3]]))
                  pm = ep.tile([128, IC, nh, 3], F32, tag="s36f", name="pm")
                  nc.vector.tensor_mul(pm, f4[:, :, :, 16:19],
                                       _bap(r, [[nh, IC], [1, nh], [0, 3]]))
                  apb = ep.tile([128, IC, nh, 3], F32, tag="s36", name="apb")
                  nc.vector.tensor_sub(apb, pm,
                                       _bap(pcam[:], [[3, IC], [0, nh], [1, 3]]))
                  sq = ep.tile([128, IC, nh, 3], F32, tag="s36b", name="sq")
                  nc.vector.tensor_mul(sq, apb, apb)
                  nc.vector.reduce_sum(out=dall[:, :, dc:dc + nh], in_=sq,
                                       axis=mybir.AxisListType.X)
                  fp = ep.tile([128, IC, nh * 3], F32, tag=f"fp{part}",
                               name="fp")
                  for ic in range(IC):
                      prod = ep.tile([128, nh, 3, 3], F32, tag="s108",
                                     name="prod", bufs=8)
                      nc.vector.tensor_mul(
                          prod,
                          _bap(apb[:, ic, :, :], [[3, nh], [0, 3], [1, 3]]),
                          _bap(frm_sb[:, ic, :], [[0, nh], [3, 3], [1, 3]]))
                      nc.vector.reduce_sum(
                          out=fp[:, ic, :].rearrange("p (x a) -> p x a", a=3),
                          in_=prod.rearrange("p h a b -> p (h a) b"),
                          axis=mybir.AxisListType.X)
                  po = 64 + hlo * 3
                  nc.vector.tensor_copy(fa1_a[:, :, po:po + nh * 3], fp)
                  fsq = ep.tile([128, IC, nh * 3], F32, tag="s36d", name="fsq")
                  nc.vector.tensor_mul(fsq, fp, fp)
                  nc.vector.reduce_sum(
                      out=dall[:, :, nc_:nc_ + nh],
                      in_=fsq.rearrange("p b (x a) -> p b x a", a=3),
                      axis=mybir.AxisListType.X)
                  defer[part] = fp

              def _emit_sqrt(c0, c1):
                  """sqrt + rsqrt of dall[:,:,c0:c1] via one ln + two exps."""
                  lnd = ep.tile([128, IC, c1 - c0], F32, tag="lnd", name="lnd")
                  nc.scalar.activation(out=lnd, in_=dall[:, :, c0:c1],
                                       func=AF.Ln, bias=tiny_sb, scale=1.0)
                  nc.scalar.activation(out=sqall[:, :, c0:c1], in_=lnd,
                                       func=AF.Exp, scale=0.5)
                  nc.scalar.activation(out=rsqall[:, :, c0:c1], in_=lnd,
                                       func=AF.Exp, scale=-0.5)

              def _emit_geo_tail(part):
                  hlo, hhi, dc, nc_ = _PARTS[part]
                  nh = hhi - hlo
                  fp = defer[part]
                  nc.vector.tensor_copy(fa1_a[:, :, 100 + hlo:100 + hhi],
                                        sqall[:, :, dc:dc + nh])
                  rn = rsqall[:, :, nc_:nc_ + 1]
                  dire = ep.tile([128, IC, nh * 3], F32, tag="s36e",
                                 name="dire")
                  nc.vector.tensor_mul(
                      dire.rearrange("p b (h a) -> p b h a", a=3),
                      fp.rearrange("p b (h a) -> p b h a", a=3),
                      _bap(rn, [[24, IC], [1, nh], [0, 3]]))
                  # dir cols 112+3*hlo .. 112+3*hhi, crossing into fa2 at 128
                  lo = 112 + 3 * hlo
                  hi = 112 + 3 * hhi
                  if lo < 128 and hi > 128:
                      nc.vector.tensor_copy(fa1_a[:, :, lo:128],
                                            dire[:, :, 0:128 - lo])
                      nc.vector.tensor_copy(fa2_a[:, :, 0:hi - 128],
                                            dire[:, :, 128 - lo:])
                  elif hi <= 128:
                      nc.vector.tensor_copy(fa1_a[:, :, lo:hi], dire)
                  else:
                      nc.vector.tensor_copy(fa2_a[:, :, lo - 128:hi - 128],
                                            dire)

              avt = {}

              def _emit_ft(g, hlf):
                  av = avt.pop((g, hlf))
                  co = (4 * g + 2 * hlf) * 20
                  nc.vector.tensor_copy(
                      ft_all[:, :, co:co + 40],
                      av[:].rearrange("p (b c) -> p b c", b=IC))

              # a "half" = one head x 512 queries x one key block: the unit
              # of both logits matmuls (N=512) and AV consumption
              halves = [(g, hlf, jb, t2)
                        for g in range(G) for hlf in range(2)
                        for jb in range(JB) for t2 in range(2)]
              # trivial: 3 halves per exp ([128,1536] = 3-bank tile, true
              # double buffering); general: 2 (uniform per-key-block bias)
              gw = 3 if trivial else 2
              groups = [halves[i:i + gw] for i in range(0, len(halves), gw)]

              backlog = []

              def _sqrt_a():
                  _emit_sqrt(0, 20)
                  _emit_geo_tail(0)
                  _emit_geo_tail("1a")
                  defer["sqA_done"] = True

              def _flush(idxs, e):
                  for q, (g, hlf, jb, t2) in enumerate(idxs):
                      first = False
                      if (g, hlf) not in avt:
                          avt[(g, hlf)] = psav.tile(
                              [128, IC * 40], F32, tag="av", name=f"av{g}{hlf}")
                          first = True
                      av = avt[(g, hlf)]
                      h = 4 * g + 2 * hlf + t2
                      for ic in range(IC):
                          nc.tensor.matmul(
                              av[:, ic * 40 + t2 * 20: ic * 40 + t2 * 20 + 20],
                              e[:, q * 512 + ic * 128: q * 512 + (ic + 1) * 128],
                              apk[:, jb, h, :],
                              start=(first and ic == 0),
                              stop=(jb == JB - 1),
                              skip_group_check=True)
                      if jb == JB - 1 and t2 == 1:
                          _emit_ft(g, hlf)
                          if (g, hlf) == (1, 1):
                              backlog.append(lambda: _emit_geo(0))
                          elif (g, hlf) == (2, 0):
                              backlog.append(lambda: _emit_geo("1a"))
                              backlog.append(_sqrt_a)

              pend = []
              for idxs in groups:
                  wide = len(idxs) * 512
                  lg = pslg.tile([128, gw * 512], F32, tag="lg", name="lg")
                  for q, (g, hlf, jb, t2) in enumerate(idxs):
                      t = 2 * hlf + t2
                      nc.tensor.matmul(
                          lg[:, q * 512:(q + 1) * 512],
                          ktp_sb[32 * t:32 * t + 16, g,
                                 jb * 128:(jb + 1) * 128],
                          qtp_sb[32 * t:32 * t + 16, g, :],
                          start=True, stop=True,
                          tile_position=(32 * t, 0))
                  e = epool.tile([128, wide], BF16, tag="E", name="e", bufs=3)
                  if trivial:
                      nc.scalar.activation(out=e, in_=lg[:, 0:wide],
                                           func=AF.Exp, scale=1.0)
                  else:
                      jb = idxs[0][2]
                      nc.scalar.activation(out=e, in_=lg[:, 0:wide],
                                           func=AF.Exp,
                                           bias=expb_sb[:, jb:jb + 1],
                                           scale=1.0)
                  # deferred geo emission: one batch per group, a group late,
                  # so its DVE inputs are long since ready and the strict-FIFO
                  # ACT (for the sqrt batch) never stalls the exp chain
                  if backlog:
                      backlog.pop(0)()
                  pend.append((idxs, e))
                  if len(pend) > 2:
                      _flush(*pend.pop(0))
              while pend:
                  _flush(*pend.pop(0))
              while backlog:
                  backlog.pop(0)()
              if "sqA_done" not in defer:
                  _sqrt_a()

              _emit_geo("1b")
              _emit_sqrt(20, 24)
              _emit_geo_tail("1b")

              # feat_all^T via transposes, then @ Wo ; residual + LN
              for ic in range(IC):
                  wo_ps = psav.tile([128, 128], F32, tag="av", name="wops")
                  fas = [(fa0_a[:, ic, :], 128), (fa1_a[:, ic, :], 128),
                         (fa2_a[:, ic, :], 32)]
                  tp = pslg.tile([128, 384], F16, tag="lg", name="tpa")
                  for cc, (fax, kk) in enumerate(fas):
                      nc.tensor.transpose(tp[0:kk, cc * 128:cc * 128 + 128],
                                          fax, identb)
                  fxt = ep.tile([128, 384], F16, tag="fxt", name="fxt")
                  nc.scalar.copy(fxt[:, 0:256], tp[:, 0:256])
                  nc.vector.tensor_copy(fxt[0:32, 256:384], tp[0:32, 256:384])
                  for cc, kk in enumerate((128, 128, 20)):
                      rhs = (wo0_sb, wo1_sb, wo2_sb)[cc]
                      nc.tensor.matmul(wo_ps[:, 0:128],
                                       fxt[0:kk, cc * 128:cc * 128 + 128], rhs,
                                       start=(cc == 0), stop=(cc == 2))
                  y = ep.tile([128, 128], F32, tag="y", name="y")
                  if trivial:
                      nc.vector.tensor_add(y, wo_ps[:, 0:128], xbo[:, ic, :])
                  else:
                      nc.vector.scalar_tensor_tensor(
                          out=y, in0=wo_ps[:, 0:128], scalar=msk[ic],
                          in1=xbo[:, ic, :], op0=ALU.mult, op1=ALU.add)
                  st6 = ep.tile([128, 6], F32, tag="st6", name="st6")
                  nc.vector.bn_stats(out=st6, in_=y)
                  mv = ep.tile([128, 2], F32, tag="mv", name="mv")
                  nc.vector.bn_aggr(out=mv, in_=st6)
                  # rstd = exp(-0.5 * ln(var + eps))
                  lnv = ep.tile([128, 1], F32, tag="lnv", name="lnv")
                  nc.scalar.activation(out=lnv, in_=mv[:, 1:2], func=AF.Ln,
                                       bias=eps_sb, scale=1.0)
                  rstd = ep.tile([128, 1], F32, tag="rstd", name="rstd")
                  nc.scalar.activation(out=rstd, in_=lnv, func=AF.Exp,
                                       scale=-0.5)
                  xc = ep.tile([128, 128], F32, tag="xc", name="xc")
                  nc.vector.tensor_scalar(out=xc, in0=y, scalar1=mv[:, 0:1],
                                          scalar2=rstd, op0=ALU.subtract,
                                          op1=ALU.mult)
                  if trivial:
                      o1 = xc
                  else:
                      o1 = ep.tile([128, 128], F32, tag="o1", name="o1")
                      nc.vector.tensor_mul(o1, xc, gmb_sb)
                      nc.vector.tensor_add(o1, o1, btb_sb)
                  eng = (nc.sync, nc.scalar, nc.gpsimd, nc.sync)[ic]
                  eng.dma_start(
                      out=out[:].rearrange("(c p) d -> c p d", p=128)[ic], in_=o1)

            for _rep in range(reps):
                _one_pass()

    # force the single ln+exp table set (ids preserved; see helper above)
    _bacc_mod.get_activation_tables = _only_ln_exp_tables
    try:
        nc.compile()
    finally:
        _bacc_mod.get_activation_tables = _orig_act_tables
    return nc


def _pm(a, nb):
    """[nb*128, F] -> partition-major [128, nb*F]."""
    f = a.shape[-1]
    return np.ascontiguousarray(
        a.reshape(nb, 128, f).transpose(1, 0, 2).reshape(128, nb * f))


def kernel(x, pos_CA, pos_CB, frame, mask, Wq, Wk, Wv, Wo, bo, gamma, beta):
    x = np.asarray(x, np.float32)
    pos_CA = np.asarray(pos_CA, np.float32)
    pos_CB = np.asarray(pos_CB, np.float32)
    frame = np.asarray(frame, np.float32)
    maskf = np.asarray(mask).astype(np.float32)
    Wq = np.asarray(Wq, np.float32)
    Wk = np.asarray(Wk, np.float32)
    Wv = np.asarray(Wv, np.float32)
    Wo = np.asarray(Wo, np.float32)
    bo = np.asarray(bo, np.float32)
    gamma = np.asarray(gamma, np.float32)
    beta = np.asarray(beta, np.float32)

    trivial = bool(
        maskf.all()
        and not bo.any()
        and (gamma == 1.0).all()
        and not beta.any()
    )
    key = ("nc", trivial)
    if key not in _compiled:
        _compiled[key] = _build(trivial=trivial)
        _compiled["nc"] = _compiled[key]
    nc = _compiled[key]
    _compiled["nc"] = nc

    wo01 = np.ascontiguousarray(np.vstack([Wo[0:256, :],]))
    wo2 = np.ascontiguousarray(Wo[256:276, :])
    bob = np.ascontiguousarray(np.tile(bo[None, :], (128, 1)))
    gmb = np.ascontiguousarray(np.tile(gamma[None, :], (128, 1)))
    btb = np.ascontiguousarray(np.tile(beta[None, :], (128, 1)))

    in_maps = []
    for c in range(NCORES):
        n, hf = c // 2, c % 2
        xn = x[n]
        sl = slice(hf * 512, (hf + 1) * 512)
        q = xn[sl] @ Wq                       # [512, 192]
        k = xn @ Wk                           # [1024, 192]
        v = xn @ Wv                           # [1024, 192]
        qtp_h = np.zeros((128, G, 512), np.float16)
        ktp_h = np.zeros((128, G, 1024), np.float16)
        for g in range(G):
            for t in range(4):
                h = 4 * g + t
                qtp_h[32 * t:32 * t + 16, g, :] = q[:, h * 16:(h + 1) * 16].T
                ktp_h[32 * t:32 * t + 16, g, :] = k[:, h * 16:(h + 1) * 16].T
        apk_h = np.ones((128, JB, H, 20), ml_dtypes.bfloat16)
        vr = v.reshape(JB, 128, H, 16).transpose(1, 0, 2, 3)
        apk_h[:, :, :, 0:16] = vr.astype(ml_dtypes.bfloat16)
        apk_h[:, :, :, 16:19] = pos_CB[n].reshape(JB, 128, 1, 3).transpose(
            1, 0, 2, 3).astype(ml_dtypes.bfloat16)
        in_maps.append({
            "qtp": qtp_h.reshape(128, G * 512),
            "ktp": ktp_h.reshape(128, G * 1024),
            "apkh": np.ascontiguousarray(apk_h.reshape(128, JB * H * 20)),
            "xq": _pm(xn[sl], 4),
            "pca": _pm(pos_CA[n, sl], 4),
            "frm": _pm(frame[n, sl].reshape(512, 9), 4),
            "expb": np.ascontiguousarray(
                (-INF * (1.0 - maskf[n])).reshape(8, 128).T),
            "mski": np.ascontiguousarray(maskf[n, sl].reshape(4, 128).T),
            "wo01": wo01.astype(np.float16),
            "wo2": wo2.astype(np.float16),
            "bob": bob, "gmb": gmb, "btb": btb,
        })

    res = bass_utils.run_bass_kernel_spmd(nc, in_maps, core_ids=list(range(NCORES)))
    full = np.empty((N, L, D), np.float32)
    for c in range(NCORES):
        n, hf = c // 2, c % 2
        full[n, hf * 512:(hf + 1) * 512, :] = res.results[c]["out"]
    return full



# revision 63
# speedup vs baseline: 10.7103x; 1.0044x over previous
"""Trainium2 Bass kernel for DDGAttention (N=4, L=1024, D=128, H=12, DQK=DV=16).

Sharding: 8 cores = 4 batch x 2 query-halves of 512. Each core runs dense
512x1024 attention for all 12 heads plus the geometric epilogue; the host
shards inputs / gathers outputs (no collectives).

Design notes (per-core):
 - q/k/v projections on the host (tiny GEMMs); device gets kT/qT pre-packed
   into 32-partition strips (head 4g+t at partitions 32t..32t+16 of group
   tensor g) and A' = [v_h | pos_CB | 1] packed per key block.
 - logits computed transposed [j, i] (lhsT = kT strip, rhs = qT strip), so
   E = exp(logits^T) feeds the AV matmul as the STATIONARY operand:
   AV out[i, c] = E_chunk^T @ A' with A' [128, 20] moving. Attention output
   lands directly in [query-partition, feature] layout -- no transposes, no
   big PSUM->SBUF copies -- and the PE streams only 20 cols per AV matmul.
 - ACT (exp over 12*512*1024 logits = ~47us busy) is the bottleneck; the
   schedule keeps it saturated: software-pipelined emission (logits matmuls
   of group n+1/n+2 before AV matmuls of group n) so the strict-FIFO PE
   never blocks the exp chain, and double-buffered [128, 1536] (3-bank)
   logit tiles amortize the per-instruction ACT overhead over 3-half spans
   (the all-ones-mask build has no per-key bias, so exps fuse across key
   blocks; the general build uses [128, 1024] spans with per-block bias).
 - softmax denominator = the ones-column of A'; rel_pos aggregation uses
   alpha @ rel_pos = alpha @ pos_CB - pos_CA * rowsum(alpha); no
   max-subtraction (logits are O(20), fp32 exp is safe).
 - every sqrt is computed as exp(0.5*ln(x)) and the ACT table list is pinned
   to natural_log_exp_and_others, so the kernel performs exactly ONE ~2.7us
   table load (no exp<->sqrt switches); distance/norm squares are batched
   into one [128, IC*24] buffer and sqrt-ed by a single ln+exp pair (a
   second tiny pair covers the last two heads in the tail).
 - geometric epilogue is emitted as head-groups complete: heads 0..7 and
   8..9 run under the remaining attention; only heads 10..11 plus the
   Wo/LayerNorm chain remain after the last exp. DVE ops are batched over
   all 4 query chunks (3-level APs) to cut per-op overhead.
 - fp16 operands on the PE-heavy paths, bf16 for E (needs fp32-range
   exponent), fp32 PSUM accumulation and fp32 residual + LayerNorm.
 - a "trivial" build (mask all-ones, bo=0, gamma=1, beta=0 -- the shipped
   setup_inputs) skips masking/affine ops and uses the fused exp spans; the
   general variant (auto-selected otherwise) keeps per-key-block exp bias.
"""

import numpy as np
import ml_dtypes

import concourse.bass as bass
import concourse.mybir as mybir
from concourse.tile import TileContext
from concourse.masks import make_identity
from concourse import bacc, bass_utils
import concourse.bacc as _bacc_mod
from concourse.hw_specs import get_activation_tables as _orig_act_tables


def _only_ln_exp_tables(arch):
    """Keep only natural_log_exp_and_others (ids preserved): the kernel uses
    exp/ln/copy exclusively, so one ACT table load suffices."""
    tabs = _orig_act_tables(arch)
    return {k: (v if k == "natural_log_exp_and_others" else set())
            for k, v in tabs.items()}


F32 = mybir.dt.float32
BF16 = mybir.dt.bfloat16
F16 = mybir.dt.float16
AF = mybir.ActivationFunctionType
ALU = mybir.AluOpType

N, L, D = 4, 1024, 128
H, DQK, DV = 12, 16, 16
NCORES = 8
JB = 8          # key blocks of 128
IC = 4          # query chunks of 128 (per 512-half)
G = 3           # head groups of 4
EPS_LN = 1e-5
INF = 1e5

# geo parts: (hlo, hhi, d2 col, n2 col) within the per-ic 24-col collector
_PARTS = {0: (0, 8, 0, 8), "1a": (8, 10, 16, 18), "1b": (10, 12, 20, 22)}

_compiled = {}


def _bap(ap, free_ap):
    """AP with replaced free dims (for 0-step broadcast reads)."""
    return bass.AP(tensor=ap.tensor, offset=ap.offset, ap=[ap.ap[0]] + free_ap)


def _build(reps=1, trivial=False):
    nc = bacc.Bacc(trn_type="TRN2")

    # ---- I/O ----------------------------------------------------------
    qtp = nc.dram_tensor("qtp", [128, G * 512], F16, kind="ExternalInput")
    ktp = nc.dram_tensor("ktp", [128, G * L], F16, kind="ExternalInput")
    apkh = nc.dram_tensor("apkh", [128, JB * H * 20], BF16, kind="ExternalInput")
    xq = nc.dram_tensor("xq", [128, IC * 128], F32, kind="ExternalInput")
    pca = nc.dram_tensor("pca", [128, IC * 3], F32, kind="ExternalInput")
    frm = nc.dram_tensor("frm", [128, IC * 9], F32, kind="ExternalInput")
    expb = nc.dram_tensor("expb", [128, JB], F32, kind="ExternalInput")
    mski = nc.dram_tensor("mski", [128, IC], F32, kind="ExternalInput")
    wo01 = nc.dram_tensor("wo01", [256, 128], F16, kind="ExternalInput")
    wo2 = nc.dram_tensor("wo2", [20, 128], F16, kind="ExternalInput")
    bob = nc.dram_tensor("bob", [128, 128], F32, kind="ExternalInput")
    gmb = nc.dram_tensor("gmb", [128, 128], F32, kind="ExternalInput")
    btb = nc.dram_tensor("btb", [128, 128], F32, kind="ExternalInput")
    out = nc.dram_tensor("out", [IC * 128, 128], F32, kind="ExternalOutput")

    with TileContext(nc) as tc:
        with tc.tile_pool(name="sing", bufs=1) as sing, \
             tc.tile_pool(name="epool", bufs=2) as epool, \
             tc.tile_pool(name="ep", bufs=4) as ep, \
             tc.tile_pool(name="pslg", bufs=2 if trivial else 3,
                          space="PSUM") as pslg, \
             tc.tile_pool(name="psav", bufs=2, space="PSUM") as psav:

            # ---- load constants / inputs (critical-path DMAs first) ---
            identb = sing.tile([128, 128], F16)
            make_identity(nc, identb)
            ktp_sb = sing.tile([128, G, L], F16)    # [16d strips, g, j]
            qtp_sb = sing.tile([128, G, 512], F16)  # [16d strips, g, i]
            apk = sing.tile([128, JB, H, 20], BF16)
            kr = ktp[:].rearrange("p (g j) -> p g j", g=G)
            qr = qtp[:].rearrange("p (g i) -> p g i", g=G)
            # two parallel critical queues: everything the first exps need
            nc.sync.dma_start(out=qtp_sb[:, 0, :], in_=qr[:, 0, :])
            nc.scalar.dma_start(out=ktp_sb[:, 0, 0:256], in_=kr[:, 0, 0:256])
            if not trivial:
                expb_sb = sing.tile([128, JB], F32)
                nc.scalar.dma_start(out=expb_sb, in_=expb[:])
            apkr = apkh[:].rearrange("p (b h c) -> p b h c", b=JB, h=H)
            nc.scalar.dma_start(out=apk[:, 0:3], in_=apkr[:, 0:3])
            nc.sync.dma_start(out=ktp_sb[:, 0, 256:768], in_=kr[:, 0, 256:768])
            nc.sync.dma_start(out=ktp_sb[:, 0, 768:1024], in_=kr[:, 0, 768:1024])
            nc.sync.dma_start(out=apk[:, 3:8], in_=apkr[:, 3:8])
            # bulk queue (gpsimd/SWDGE): later groups + epilogue inputs
            for _g in (1, 2):
                nc.gpsimd.dma_start(out=ktp_sb[:, _g, :], in_=kr[:, _g, :])
                nc.gpsimd.dma_start(out=qtp_sb[:, _g, :], in_=qr[:, _g, :])
            xq_sb = sing.tile([128, IC, 128], F32)
            nc.gpsimd.dma_start(out=xq_sb,
                                in_=xq[:].rearrange("p (b d) -> p b d", b=IC))
            pca_sb = sing.tile([128, IC, 3], F32)
            nc.gpsimd.dma_start(out=pca_sb,
                                in_=pca[:].rearrange("p (b c) -> p b c", b=IC))
            frm_sb = sing.tile([128, IC, 9], F32)
            nc.gpsimd.dma_start(out=frm_sb,
                                in_=frm[:].rearrange("p (b c) -> p b c", b=IC))
            mski_sb = sing.tile([128, IC], F32)
            nc.gpsimd.dma_start(out=mski_sb, in_=mski[:])
            wo0_sb = sing.tile([128, 128], F16)
            nc.gpsimd.dma_start(out=wo0_sb, in_=wo01[0:128, :])
            wo1_sb = sing.tile([128, 128], F16)
            nc.gpsimd.dma_start(out=wo1_sb, in_=wo01[128:256, :])
            wo2_sb = sing.tile([20, 128], F16)
            nc.gpsimd.dma_start(out=wo2_sb, in_=wo2[:])
            bob_sb = sing.tile([128, 128], F32)
            nc.gpsimd.dma_start(out=bob_sb, in_=bob[:])
            gmb_sb = sing.tile([128, 128], F32)
            nc.gpsimd.dma_start(out=gmb_sb, in_=gmb[:])
            btb_sb = sing.tile([128, 128], F32)
            nc.gpsimd.dma_start(out=btb_sb, in_=btb[:])
            eps_sb = sing.tile([128, 1], F32)
            nc.vector.memset(eps_sb, EPS_LN)
            tiny_sb = sing.tile([128, 1], F32)
            nc.vector.memset(tiny_sb, 1e-30)
            warm = sing.tile([128, 1], F32)
            nc.scalar.activation(out=warm, in_=eps_sb, func=AF.Exp)
            # PE warm-up during the input-DMA wait: dummy matmuls on a
            # DVE-zeroed tile (ready at ~0.1us -- independent of DMAs and of
            # the gpsimd identity build) release the HAM clock throttle so
            # the first real matmuls run at full rate. Sized to finish before
            # the first inputs land, so they never delay real work.
            wz = sing.tile([128, 128], F16)
            nc.vector.memset(wz, 0.0)
            pewu = pslg.tile([128, 128], F32, tag="lg", name="pewu")
            for _ in range(14):
                nc.tensor.matmul(pewu, wz, wz, start=True, stop=True)

            # (reps>1 only for replication-slope timing)
            def _one_pass():
              if trivial:
                  xbo = xq_sb
              else:
                  xbo = sing.tile([128, IC, 128], F32)
                  for ic in range(IC):
                      nc.vector.scalar_tensor_tensor(
                          out=xbo[:, ic, :], in0=bob_sb,
                          scalar=mski_sb[:, ic:ic + 1],
                          in1=xq_sb[:, ic, :], op0=ALU.mult, op1=ALU.add)

              # attention output, [i-part, ic, h*20+c] (c: 16 v | 3 pos | den)
              ft_all = sing.tile([128, IC, H * 20], F32)
              # distance/norm squares collector (see _PARTS)
              dall = sing.tile([128, IC, 24], F32)
              sqall = sing.tile([128, IC, 24], F32)    # sqrt(dall)
              rsqall = sing.tile([128, IC, 24], F32)   # rsqrt(dall)
              # feat staging [i, ic, c] (f16 for the Wo transposes)
              fa0_a = sing.tile([128, IC, 128], F16)
              fa1_a = sing.tile([128, IC, 128], F16)
              fa2_a = sing.tile([128, IC, 32], F16)
              nc.vector.memset(fa2_a[:, :, 20:32], 0.0)

              msk = [mski_sb[:, ic:ic + 1] for ic in range(IC)]
              defer = {}

              def _emit_geo(part):
                  hlo, hhi, dc, nc_ = _PARTS[part]
                  nh = hhi - hlo
                  f4 = ft_all[:, :, hlo * 20:hhi * 20].rearrange(
                      "p b (h c) -> p b h c", c=20)
                  r = ep.tile([128, IC, nh], F32, tag="s12b", name="rden")
                  nc.vector.reciprocal(
                      r, f4[:, :, :, 19:20].rearrange("p b h o -> p b (h o)"))
                  if not trivial:
                      r2 = ep.tile([128, IC, nh], F32, tag="s12c", name="rm")
                      nc.vector.tensor_mul(
                          r2, r, _bap(mski_sb[:], [[1, IC], [0, nh]]))
                      r = r2
                  if part == 0:
                      node_dst = fa0_a
                  else:
                      nb = (hlo - 8) * 16
                      node_dst = fa1_a[:, :, nb:nb + nh * 16]
                  nc.vector.tensor_mul(
                      node_dst.rearrange("p b (h c) -> p b h c", c=16),
                      f4[:, :, :, 0:16], _bap(r, [[nh, IC], [1, nh], [0, 16]]))
                  if trivial:
                      pcam = pca_sb
                  else:
                      pcam = ep.tile([128, IC, 3], F32, tag="s3", name="pcam")
                      nc.vector.tensor_mul(
                          pcam, pca_sb, _bap(mski_sb[:], [[1, IC], [0, 3]]))
                  pm = ep.tile([128, IC, nh, 3], F32, tag="s36f", name="pm")
                  nc.vector.tensor_mul(pm, f4[:, :, :, 16:19],
                                       _bap(r, [[nh, IC], [1, nh], [0, 3]]))
                  apb = ep.tile([128, IC, nh, 3], F32, tag="s36", name="apb")
                  nc.vector.tensor_sub(apb, pm,
                                       _bap(pcam[:], [[3, IC], [0, nh], [1, 3]]))
                  sq = ep.tile([128, IC, nh, 3], F32, tag="s36b", name="sq")
                  nc.vector.tensor_mul(sq, apb, apb)
                  nc.vector.reduce_sum(out=dall[:, :, dc:dc + nh], in_=sq,
                                       axis=mybir.AxisListType.X)
                  fp = ep.tile([128, IC, nh * 3], F32, tag=f"fp{part}",
                               name="fp")
                  if nh == 2:
                      # IC-batched per-head frame product: halves the DVE op
                      # count on the tail-critical heads-10/11 chain
                      for h in range(nh):
                          prod2 = ep.tile([128, IC, 3, 3], F32, tag="s108b",
                                          name="prod2", bufs=4)
                          nc.vector.tensor_mul(
                              prod2,
                              _bap(apb[:, :, h, :],
                                   [[nh * 3, IC], [0, 3], [1, 3]]),
                              frm_sb[:].rearrange("p b (a c) -> p b a c", a=3))
                          nc.vector.reduce_sum(
                              out=fp[:, :, h * 3:h * 3 + 3], in_=prod2,
                              axis=mybir.AxisListType.X)
                  else:
                    for ic in range(IC):
                      prod = ep.tile([128, nh, 3, 3], F32, tag="s108",
                                     name="prod", bufs=8)
                      nc.vector.tensor_mul(
                          prod,
                          _bap(apb[:, ic, :, :], [[3, nh], [0, 3], [1, 3]]),
                          _bap(frm_sb[:, ic, :], [[0, nh], [3, 3], [1, 3]]))
                      nc.vector.reduce_sum(
                          out=fp[:, ic, :].rearrange("p (x a) -> p x a", a=3),
                          in_=prod.rearrange("p h a b -> p (h a) b"),
                          axis=mybir.AxisListType.X)
                  po = 64 + hlo * 3
                  nc.vector.tensor_copy(fa1_a[:, :, po:po + nh * 3], fp)
                  fsq = ep.tile([128, IC, nh * 3], F32, tag="s36d", name="fsq")
                  nc.vector.tensor_mul(fsq, fp, fp)
                  nc.vector.reduce_sum(
                      out=dall[:, :, nc_:nc_ + nh],
                      in_=fsq.rearrange("p b (x a) -> p b x a", a=3),
                      axis=mybir.AxisListType.X)
                  defer[part] = fp

              def _emit_sqrt(c0, c1):
                  """sqrt + rsqrt of dall[:,:,c0:c1] via one ln + two exps."""
                  lnd = ep.tile([128, IC, c1 - c0], F32, tag="lnd", name="lnd")
                  nc.scalar.activation(out=lnd, in_=dall[:, :, c0:c1],
                                       func=AF.Ln, bias=tiny_sb, scale=1.0)
                  nc.scalar.activation(out=sqall[:, :, c0:c1], in_=lnd,
                                       func=AF.Exp, scale=0.5)
                  nc.scalar.activation(out=rsqall[:, :, c0:c1], in_=lnd,
                                       func=AF.Exp, scale=-0.5)

              def _emit_geo_tail(part):
                  hlo, hhi, dc, nc_ = _PARTS[part]
                  nh = hhi - hlo
                  fp = defer[part]
                  nc.vector.tensor_copy(fa1_a[:, :, 100 + hlo:100 + hhi],
                                        sqall[:, :, dc:dc + nh])
                  rn = rsqall[:, :, nc_:nc_ + 1]
                  dire = ep.tile([128, IC, nh * 3], F32, tag="s36e",
                                 name="dire")
                  nc.vector.tensor_mul(
                      dire.rearrange("p b (h a) -> p b h a", a=3),
                      fp.rearrange("p b (h a) -> p b h a", a=3),
                      _bap(rn, [[24, IC], [1, nh], [0, 3]]))
                  # dir cols 112+3*hlo .. 112+3*hhi, crossing into fa2 at 128
                  lo = 112 + 3 * hlo
                  hi = 112 + 3 * hhi
                  if lo < 128 and hi > 128:
                      nc.vector.tensor_copy(fa1_a[:, :, lo:128],
                                            dire[:, :, 0:128 - lo])
                      nc.vector.tensor_copy(fa2_a[:, :, 0:hi - 128],
                                            dire[:, :, 128 - lo:])
                  elif hi <= 128:
                      nc.vector.tensor_copy(fa1_a[:, :, lo:hi], dire)
                  else:
                      nc.vector.tensor_copy(fa2_a[:, :, lo - 128:hi - 128],
                                            dire)

              avt = {}

              def _emit_ft(g, hlf):
                  av = avt.pop((g, hlf))
                  co = (4 * g + 2 * hlf) * 20
                  nc.vector.tensor_copy(
                      ft_all[:, :, co:co + 40],
                      av[:].rearrange("p (b c) -> p b c", b=IC))

              # a "half" = one head x 512 queries x one key block: the unit
              # of both logits matmuls (N=512) and AV consumption
              halves = [(g, hlf, jb, t2)
                        for g in range(G) for hlf in range(2)
                        for jb in range(JB) for t2 in range(2)]
              # trivial: 3 halves per exp ([128,1536] = 3-bank tile, true
              # double buffering); general: 2 (uniform per-key-block bias)
              gw = 3 if trivial else 2
              groups = [halves[i:i + gw] for i in range(0, len(halves), gw)]

              backlog = []

              def _sqrt_a():
                  _emit_sqrt(0, 20)
                  _emit_geo_tail(0)
                  _emit_geo_tail("1a")
                  defer["sqA_done"] = True

              def _flush(idxs, e):
                  for q, (g, hlf, jb, t2) in enumerate(idxs):
                      first = False
                      if (g, hlf) not in avt:
                          avt[(g, hlf)] = psav.tile(
                              [128, IC * 40], F32, tag="av", name=f"av{g}{hlf}")
                          first = True
                      av = avt[(g, hlf)]
                      h = 4 * g + 2 * hlf + t2
                      for ic in range(IC):
                          nc.tensor.matmul(
                              av[:, ic * 40 + t2 * 20: ic * 40 + t2 * 20 + 20],
                              e[:, q * 512 + ic * 128: q * 512 + (ic + 1) * 128],
                              apk[:, jb, h, :],
                              start=(first and ic == 0),
                              stop=(jb == JB - 1),
                              skip_group_check=True)
                      if jb == JB - 1 and t2 == 1:
                          _emit_ft(g, hlf)
                          if (g, hlf) == (1, 1):
                              backlog.append(lambda: _emit_geo(0))
                          elif (g, hlf) == (2, 0):
                              backlog.append(lambda: _emit_geo("1a"))
                              backlog.append(_sqrt_a)

              pend = []
              for idxs in groups:
                  wide = len(idxs) * 512
                  lg = pslg.tile([128, gw * 512], F32, tag="lg", name="lg")
                  for q, (g, hlf, jb, t2) in enumerate(idxs):
                      t = 2 * hlf + t2
                      nc.tensor.matmul(
                          lg[:, q * 512:(q + 1) * 512],
                          ktp_sb[32 * t:32 * t + 16, g,
                                 jb * 128:(jb + 1) * 128],
                          qtp_sb[32 * t:32 * t + 16, g, :],
                          start=True, stop=True,
                          tile_position=(32 * t, 0))
                  e = epool.tile([128, wide], BF16, tag="E", name="e", bufs=3)
                  if trivial:
                      nc.scalar.activation(out=e, in_=lg[:, 0:wide],
                                           func=AF.Exp, scale=1.0)
                  else:
                      jb = idxs[0][2]
                      nc.scalar.activation(out=e, in_=lg[:, 0:wide],
                                           func=AF.Exp,
                                           bias=expb_sb[:, jb:jb + 1],
                                           scale=1.0)
                  # deferred geo emission: one batch per group, a group late,
                  # so its DVE inputs are long since ready and the strict-FIFO
                  # ACT (for the sqrt batch) never stalls the exp chain
                  if backlog:
                      backlog.pop(0)()
                  pend.append((idxs, e))
                  if len(pend) > 2:
                      _flush(*pend.pop(0))
              while pend:
                  _flush(*pend.pop(0))
              while backlog:
                  backlog.pop(0)()
              if "sqA_done" not in defer:
                  _sqrt_a()

              _emit_geo("1b")
              _emit_sqrt(20, 24)
              _emit_geo_tail("1b")

              # feat_all^T via transposes, then @ Wo ; residual + LN
              for ic in range(IC):
                  wo_ps = psav.tile([128, 128], F32, tag="av", name="wops")
                  fas = [(fa0_a[:, ic, :], 128), (fa1_a[:, ic, :], 128),
                         (fa2_a[:, ic, :], 32)]
                  tp = pslg.tile([128, 384], F16, tag="lg", name="tpa")
                  for cc, (fax, kk) in enumerate(fas):
                      nc.tensor.transpose(tp[0:kk, cc * 128:cc * 128 + 128],
                                          fax, identb)
                  fxt = ep.tile([128, 384], F16, tag="fxt", name="fxt")
                  nc.scalar.copy(fxt[:, 0:256], tp[:, 0:256])
                  nc.vector.tensor_copy(fxt[0:32, 256:384], tp[0:32, 256:384])
                  for cc, kk in enumerate((128, 128, 20)):
                      rhs = (wo0_sb, wo1_sb, wo2_sb)[cc]
                      nc.tensor.matmul(wo_ps[:, 0:128],
                                       fxt[0:kk, cc * 128:cc * 128 + 128], rhs,
                                       start=(cc == 0), stop=(cc == 2))
                  y = ep.tile([128, 128], F32, tag="y", name="y")
                  if trivial:
                      nc.vector.tensor_add(y, wo_ps[:, 0:128], xbo[:, ic, :])
                  else:
                      nc.vector.scalar_tensor_tensor(
                          out=y, in0=wo_ps[:, 0:128], scalar=msk[ic],
                          in1=xbo[:, ic, :], op0=ALU.mult, op1=ALU.add)
                  st6 = ep.tile([128, 6], F32, tag="st6", name="st6")
                  nc.vector.bn_stats(out=st6, in_=y)
                  mv = ep.tile([128, 2], F32, tag="mv", name="mv")
                  nc.vector.bn_aggr(out=mv, in_=st6)
                  # rstd = exp(-0.5 * ln(var + eps))
                  lnv = ep.tile([128, 1], F32, tag="lnv", name="lnv")
                  nc.scalar.activation(out=lnv, in_=mv[:, 1:2], func=AF.Ln,
                                       bias=eps_sb, scale=1.0)
                  rstd = ep.tile([128, 1], F32, tag="rstd", name="rstd")
                  nc.scalar.activation(out=rstd, in_=lnv, func=AF.Exp,
                                       scale=-0.5)
                  xc = ep.tile([128, 128], F32, tag="xc", name="xc")
                  nc.vector.tensor_scalar(out=xc, in0=y, scalar1=mv[:, 0:1],
                                          scalar2=rstd, op0=ALU.subtract,
                                          op1=ALU.mult)
                  if trivial:
                      o1 = xc
                  else:
                      o1 = ep.tile([128, 128], F32, tag="o1", name="o1")
                      nc.vector.tensor_mul(o1, xc, gmb_sb)
                      nc.vector.tensor_add(o1, o1, btb_sb)
                  eng = (nc.sync, nc.scalar, nc.sync, nc.scalar)[ic]
                  eng.dma_start(
                      out=out[:].rearrange("(c p) d -> c p d", p=128)[ic], in_=o1)

            for _rep in range(reps):
                _one_pass()

    # force the single ln+exp table set (ids preserved; see helper above)
    _bacc_mod.get_activation_tables = _only_ln_exp_tables
    try:
        nc.compile()
    finally:
        _bacc_mod.get_activation_tables = _orig_act_tables
    return nc


def _pm(a, nb):
    """[nb*128, F] -> partition-major [128, nb*F]."""
    f = a.shape[-1]
    return np.ascontiguousarray(
        a.reshape(nb, 128, f).transpose(1, 0, 2).reshape(128, nb * f))


def kernel(x, pos_CA, pos_CB, frame, mask, Wq, Wk, Wv, Wo, bo, gamma, beta):
    x = np.asarray(x, np.float32)
    pos_CA = np.asarray(pos_CA, np.float32)
    pos_CB = np.asarray(pos_CB, np.float32)
    frame = np.asarray(frame, np.float32)
    maskf = np.asarray(mask).astype(np.float32)
    Wq = np.asarray(Wq, np.float32)
    Wk = np.asarray(Wk, np.float32)
    Wv = np.asarray(Wv, np.float32)
    Wo = np.asarray(Wo, np.float32)
    bo = np.asarray(bo, np.float32)
    gamma = np.asarray(gamma, np.float32)
    beta = np.asarray(beta, np.float32)

    trivial = bool(
        maskf.all()
        and not bo.any()
        and (gamma == 1.0).all()
        and not beta.any()
    )
    key = ("nc", trivial)
    if key not in _compiled:
        _compiled[key] = _build(trivial=trivial)
        _compiled["nc"] = _compiled[key]
    nc = _compiled[key]
    _compiled["nc"] = nc

    wo01 = np.ascontiguousarray(np.vstack([Wo[0:256, :],]))
    wo2 = np.ascontiguousarray(Wo[256:276, :])
    bob = np.ascontiguousarray(np.tile(bo[None, :], (128, 1)))
    gmb = np.ascontiguousarray(np.tile(gamma[None, :], (128, 1)))
    btb = np.ascontiguousarray(np.tile(beta[None, :], (128, 1)))

    in_maps = []
    for c in range(NCORES):
        n, hf = c // 2, c % 2
        xn = x[n]
        sl = slice(hf * 512, (hf + 1) * 512)
        q = xn[sl] @ Wq                       # [512, 192]
        k = xn @ Wk                           # [1024, 192]
        v = xn @ Wv                           # [1024, 192]
        qtp_h = np.zeros((128, G, 512), np.float16)
        ktp_h = np.zeros((128, G, 1024), np.float16)
        for g in range(G):
            for t in range(4):
                h = 4 * g + t
                qtp_h[32 * t:32 * t + 16, g, :] = q[:, h * 16:(h + 1) * 16].T
                ktp_h[32 * t:32 * t + 16, g, :] = k[:, h * 16:(h + 1) * 16].T
        apk_h = np.ones((128, JB, H, 20), ml_dtypes.bfloat16)
        vr = v.reshape(JB, 128, H, 16).transpose(1, 0, 2, 3)
        apk_h[:, :, :, 0:16] = vr.astype(ml_dtypes.bfloat16)
        apk_h[:, :, :, 16:19] = pos_CB[n].reshape(JB, 128, 1, 3).transpose(
            1, 0, 2, 3).astype(ml_dtypes.bfloat16)
        in_maps.append({
            "qtp": qtp_h.reshape(128, G * 512),
            "ktp": ktp_h.reshape(128, G * 1024),
            "apkh": np.ascontiguousarray(apk_h.reshape(128, JB * H * 20)),
            "xq": _pm(xn[sl], 4),
            "pca": _pm(pos_CA[n, sl], 4),
            "frm": _pm(frame[n, sl].reshape(512, 9), 4),
            "expb": np.ascontiguousarray(
                (-INF * (1.0 - maskf[n])).reshape(8, 128).T),
            "mski": np.ascontiguousarray(maskf[n, sl].reshape(4, 128).T),
            "wo01": wo01.astype(np.float16),
            "wo2": wo2.astype(np.float16),
            "bob": bob, "gmb": gmb, "btb": btb,
        })

    res = bass_utils.run_bass_kernel_spmd(nc, in_maps, core_ids=list(range(NCORES)))
    full = np.empty((N, L, D), np.float32)
    for c in range(NCORES):
        n, hf = c // 2, c % 2
        full[n, hf * 512:(hf + 1) * 512, :] = res.results[c]["out"]
    return full



# revision 64
# speedup vs baseline: 10.7355x; 1.0024x over previous
"""Trainium2 Bass kernel for DDGAttention (N=4, L=1024, D=128, H=12, DQK=DV=16).

Sharding: 8 cores = 4 batch x 2 query-halves of 512. Each core runs dense
512x1024 attention for all 12 heads plus the geometric epilogue; the host
shards inputs / gathers outputs (no collectives).

Design notes (per-core):
 - q/k/v projections on the host (tiny GEMMs); device gets kT/qT pre-packed
   into 32-partition strips (head 4g+t at partitions 32t..32t+16 of group
   tensor g) and A' = [v_h | pos_CB | 1] packed per key block.
 - logits computed transposed [j, i] (lhsT = kT strip, rhs = qT strip), so
   E = exp(logits^T) feeds the AV matmul as the STATIONARY operand:
   AV out[i, c] = E_chunk^T @ A' with A' [128, 20] moving. Attention output
   lands directly in [query-partition, feature] layout -- no transposes, no
   big PSUM->SBUF copies -- and the PE streams only 20 cols per AV matmul.
 - ACT (exp over 12*512*1024 logits = ~47us busy) is the bottleneck; the
   schedule keeps it saturated: software-pipelined emission (logits matmuls
   of group n+1/n+2 before AV matmuls of group n) so the strict-FIFO PE
   never blocks the exp chain, and double-buffered [128, 1536] (3-bank)
   logit tiles amortize the per-instruction ACT overhead over 3-half spans
   (the all-ones-mask build has no per-key bias, so exps fuse across key
   blocks; the general build uses [128, 1024] spans with per-block bias).
 - softmax denominator = the ones-column of A'; rel_pos aggregation uses
   alpha @ rel_pos = alpha @ pos_CB - pos_CA * rowsum(alpha); no
   max-subtraction (logits are O(20), fp32 exp is safe).
 - every sqrt is computed as exp(0.5*ln(x)) and the ACT table list is pinned
   to natural_log_exp_and_others, so the kernel performs exactly ONE ~2.7us
   table load (no exp<->sqrt switches); distance/norm squares are batched
   into one [128, IC*24] buffer and sqrt-ed by a single ln+exp pair (a
   second tiny pair covers the last two heads in the tail).
 - geometric epilogue is emitted as head-groups complete: heads 0..7 and
   8..9 run under the remaining attention; only heads 10..11 plus the
   Wo/LayerNorm chain remain after the last exp. DVE ops are batched over
   all 4 query chunks (3-level APs) to cut per-op overhead.
 - fp16 operands on the PE-heavy paths, bf16 for E (needs fp32-range
   exponent), fp32 PSUM accumulation and fp32 residual + LayerNorm.
 - a "trivial" build (mask all-ones, bo=0, gamma=1, beta=0 -- the shipped
   setup_inputs) skips masking/affine ops and uses the fused exp spans; the
   general variant (auto-selected otherwise) keeps per-key-block exp bias.
"""

import numpy as np
import ml_dtypes

import concourse.bass as bass
import concourse.mybir as mybir
from concourse.tile import TileContext
from concourse.masks import make_identity
from concourse import bacc, bass_utils
import concourse.bacc as _bacc_mod
from concourse.hw_specs import get_activation_tables as _orig_act_tables


def _only_ln_exp_tables(arch):
    """Keep only natural_log_exp_and_others (ids preserved): the kernel uses
    exp/ln/copy exclusively, so one ACT table load suffices."""
    tabs = _orig_act_tables(arch)
    return {k: (v if k == "natural_log_exp_and_others" else set())
            for k, v in tabs.items()}


F32 = mybir.dt.float32
BF16 = mybir.dt.bfloat16
F16 = mybir.dt.float16
AF = mybir.ActivationFunctionType
ALU = mybir.AluOpType

N, L, D = 4, 1024, 128
H, DQK, DV = 12, 16, 16
NCORES = 8
JB = 8          # key blocks of 128
IC = 4          # query chunks of 128 (per 512-half)
G = 3           # head groups of 4
EPS_LN = 1e-5
INF = 1e5

# geo parts: (hlo, hhi, d2 col, n2 col) within the per-ic 24-col collector
_PARTS = {0: (0, 8, 0, 8), "1a": (8, 10, 16, 18), "1b": (10, 12, 20, 22)}

_compiled = {}


def _bap(ap, free_ap):
    """AP with replaced free dims (for 0-step broadcast reads)."""
    return bass.AP(tensor=ap.tensor, offset=ap.offset, ap=[ap.ap[0]] + free_ap)


def _build(reps=1, trivial=False):
    nc = bacc.Bacc(trn_type="TRN2")

    # ---- I/O ----------------------------------------------------------
    qtp = nc.dram_tensor("qtp", [128, G * 512], F16, kind="ExternalInput")
    ktp = nc.dram_tensor("ktp", [128, G * L], F16, kind="ExternalInput")
    apkh = nc.dram_tensor("apkh", [128, JB * H * 20], BF16, kind="ExternalInput")
    xq = nc.dram_tensor("xq", [128, IC * 128], F32, kind="ExternalInput")
    pca = nc.dram_tensor("pca", [128, IC * 3], F32, kind="ExternalInput")
    frm = nc.dram_tensor("frm", [128, IC * 9], F32, kind="ExternalInput")
    expb = nc.dram_tensor("expb", [128, JB], F32, kind="ExternalInput")
    mski = nc.dram_tensor("mski", [128, IC], F32, kind="ExternalInput")
    wo01 = nc.dram_tensor("wo01", [256, 128], F16, kind="ExternalInput")
    wo2 = nc.dram_tensor("wo2", [20, 128], F16, kind="ExternalInput")
    bob = nc.dram_tensor("bob", [128, 128], F32, kind="ExternalInput")
    gmb = nc.dram_tensor("gmb", [128, 128], F32, kind="ExternalInput")
    btb = nc.dram_tensor("btb", [128, 128], F32, kind="ExternalInput")
    out = nc.dram_tensor("out", [IC * 128, 128], F32, kind="ExternalOutput")

    with TileContext(nc) as tc:
        with tc.tile_pool(name="sing", bufs=1) as sing, \
             tc.tile_pool(name="epool", bufs=2) as epool, \
             tc.tile_pool(name="ep", bufs=4) as ep, \
             tc.tile_pool(name="pslg", bufs=2 if trivial else 3,
                          space="PSUM") as pslg, \
             tc.tile_pool(name="psav", bufs=2, space="PSUM") as psav:

            # ---- load constants / inputs (critical-path DMAs first) ---
            identb = sing.tile([128, 128], F16)
            make_identity(nc, identb)
            ktp_sb = sing.tile([128, G, L], F16)    # [16d strips, g, j]
            qtp_sb = sing.tile([128, G, 512], F16)  # [16d strips, g, i]
            apk = sing.tile([128, JB, H, 20], BF16)
            kr = ktp[:].rearrange("p (g j) -> p g j", g=G)
            qr = qtp[:].rearrange("p (g i) -> p g i", g=G)
            # two parallel critical queues: everything the first exps need
            nc.sync.dma_start(out=qtp_sb[:, 0, :], in_=qr[:, 0, :])
            nc.scalar.dma_start(out=ktp_sb[:, 0, 0:256], in_=kr[:, 0, 0:256])
            if not trivial:
                expb_sb = sing.tile([128, JB], F32)
                nc.scalar.dma_start(out=expb_sb, in_=expb[:])
            apkr = apkh[:].rearrange("p (b h c) -> p b h c", b=JB, h=H)
            nc.scalar.dma_start(out=apk[:, 0:3], in_=apkr[:, 0:3])
            nc.sync.dma_start(out=ktp_sb[:, 0, 256:768], in_=kr[:, 0, 256:768])
            nc.sync.dma_start(out=ktp_sb[:, 0, 768:1024], in_=kr[:, 0, 768:1024])
            nc.sync.dma_start(out=apk[:, 3:8], in_=apkr[:, 3:8])
            # bulk queue (gpsimd/SWDGE): later groups + epilogue inputs
            for _g in (1, 2):
                nc.gpsimd.dma_start(out=ktp_sb[:, _g, :], in_=kr[:, _g, :])
                nc.gpsimd.dma_start(out=qtp_sb[:, _g, :], in_=qr[:, _g, :])
            xq_sb = sing.tile([128, IC, 128], F32)
            nc.gpsimd.dma_start(out=xq_sb,
                                in_=xq[:].rearrange("p (b d) -> p b d", b=IC))
            pca_sb = sing.tile([128, IC, 3], F32)
            nc.gpsimd.dma_start(out=pca_sb,
                                in_=pca[:].rearrange("p (b c) -> p b c", b=IC))
            frm_sb = sing.tile([128, IC, 9], F32)
            nc.gpsimd.dma_start(out=frm_sb,
                                in_=frm[:].rearrange("p (b c) -> p b c", b=IC))
            mski_sb = sing.tile([128, IC], F32)
            nc.gpsimd.dma_start(out=mski_sb, in_=mski[:])
            wo0_sb = sing.tile([128, 128], F16)
            nc.gpsimd.dma_start(out=wo0_sb, in_=wo01[0:128, :])
            wo1_sb = sing.tile([128, 128], F16)
            nc.gpsimd.dma_start(out=wo1_sb, in_=wo01[128:256, :])
            wo2_sb = sing.tile([20, 128], F16)
            nc.gpsimd.dma_start(out=wo2_sb, in_=wo2[:])
            bob_sb = sing.tile([128, 128], F32)
            nc.gpsimd.dma_start(out=bob_sb, in_=bob[:])
            gmb_sb = sing.tile([128, 128], F32)
            nc.gpsimd.dma_start(out=gmb_sb, in_=gmb[:])
            btb_sb = sing.tile([128, 128], F32)
            nc.gpsimd.dma_start(out=btb_sb, in_=btb[:])
            eps_sb = sing.tile([128, 1], F32)
            nc.vector.memset(eps_sb, EPS_LN)
            tiny_sb = sing.tile([128, 1], F32)
            nc.vector.memset(tiny_sb, 1e-30)
            warm = sing.tile([128, 1], F32)
            nc.scalar.activation(out=warm, in_=eps_sb, func=AF.Exp)
            # PE warm-up during the input-DMA wait: dummy matmuls on a
            # DVE-zeroed tile (ready at ~0.1us -- independent of DMAs and of
            # the gpsimd identity build) release the HAM clock throttle so
            # the first real matmuls run at full rate. Sized to finish before
            # the first inputs land, so they never delay real work.
            wz = sing.tile([128, 128], F16)
            nc.vector.memset(wz, 0.0)
            pewu = pslg.tile([128, 128], F32, tag="lg", name="pewu")
            for _ in range(14):
                nc.tensor.matmul(pewu, wz, wz, start=True, stop=True)

            # (reps>1 only for replication-slope timing)
            def _one_pass():
              if trivial:
                  xbo = xq_sb
              else:
                  xbo = sing.tile([128, IC, 128], F32)
                  for ic in range(IC):
                      nc.vector.scalar_tensor_tensor(
                          out=xbo[:, ic, :], in0=bob_sb,
                          scalar=mski_sb[:, ic:ic + 1],
                          in1=xq_sb[:, ic, :], op0=ALU.mult, op1=ALU.add)

              # attention output, [i-part, ic, h*20+c] (c: 16 v | 3 pos | den)
              ft_all = sing.tile([128, IC, H * 20], F32)
              # distance/norm squares collector (see _PARTS)
              dall = sing.tile([128, IC, 24], F32)
              sqall = sing.tile([128, IC, 24], F32)    # sqrt(dall)
              rsqall = sing.tile([128, IC, 24], F32)   # rsqrt(dall)
              # feat staging [i, ic, c] (f16 for the Wo transposes)
              fa0_a = sing.tile([128, IC, 128], F16)
              fxt0_sb = sing.tile([128, IC, 128], F16)  # fa0^T via XBAR
              fa1_a = sing.tile([128, IC, 128], F16)
              fa2_a = sing.tile([128, IC, 32], F16)
              nc.vector.memset(fa2_a[:, :, 20:32], 0.0)

              msk = [mski_sb[:, ic:ic + 1] for ic in range(IC)]
              defer = {}

              def _emit_geo(part):
                  hlo, hhi, dc, nc_ = _PARTS[part]
                  nh = hhi - hlo
                  f4 = ft_all[:, :, hlo * 20:hhi * 20].rearrange(
                      "p b (h c) -> p b h c", c=20)
                  r = ep.tile([128, IC, nh], F32, tag="s12b", name="rden")
                  nc.vector.reciprocal(
                      r, f4[:, :, :, 19:20].rearrange("p b h o -> p b (h o)"))
                  if not trivial:
                      r2 = ep.tile([128, IC, nh], F32, tag="s12c", name="rm")
                      nc.vector.tensor_mul(
                          r2, r, _bap(mski_sb[:], [[1, IC], [0, nh]]))
                      r = r2
                  if part == 0:
                      node_dst = fa0_a
                  else:
                      nb = (hlo - 8) * 16
                      node_dst = fa1_a[:, :, nb:nb + nh * 16]
                  nc.vector.tensor_mul(
                      node_dst.rearrange("p b (h c) -> p b h c", c=16),
                      f4[:, :, :, 0:16], _bap(r, [[nh, IC], [1, nh], [0, 16]]))
                  if trivial:
                      pcam = pca_sb
                  else:
                      pcam = ep.tile([128, IC, 3], F32, tag="s3", name="pcam")
                      nc.vector.tensor_mul(
                          pcam, pca_sb, _bap(mski_sb[:], [[1, IC], [0, 3]]))
                  pm = ep.tile([128, IC, nh, 3], F32, tag="s36f", name="pm")
                  nc.vector.tensor_mul(pm, f4[:, :, :, 16:19],
                                       _bap(r, [[nh, IC], [1, nh], [0, 3]]))
                  apb = ep.tile([128, IC, nh, 3], F32, tag="s36", name="apb")
                  nc.vector.tensor_sub(apb, pm,
                                       _bap(pcam[:], [[3, IC], [0, nh], [1, 3]]))
                  sq = ep.tile([128, IC, nh, 3], F32, tag="s36b", name="sq")
                  nc.vector.tensor_mul(sq, apb, apb)
                  nc.vector.reduce_sum(out=dall[:, :, dc:dc + nh], in_=sq,
                                       axis=mybir.AxisListType.X)
                  fp = ep.tile([128, IC, nh * 3], F32, tag=f"fp{part}",
                               name="fp")
                  if nh == 2:
                      # IC-batched per-head frame product: halves the DVE op
                      # count on the tail-critical heads-10/11 chain
                      for h in range(nh):
                          prod2 = ep.tile([128, IC, 3, 3], F32, tag="s108b",
                                          name="prod2", bufs=4)
                          nc.vector.tensor_mul(
                              prod2,
                              _bap(apb[:, :, h, :],
                                   [[nh * 3, IC], [0, 3], [1, 3]]),
                              frm_sb[:].rearrange("p b (a c) -> p b a c", a=3))
                          nc.vector.reduce_sum(
                              out=fp[:, :, h * 3:h * 3 + 3], in_=prod2,
                              axis=mybir.AxisListType.X)
                  else:
                    for ic in range(IC):
                      prod = ep.tile([128, nh, 3, 3], F32, tag="s108",
                                     name="prod", bufs=8)
                      nc.vector.tensor_mul(
                          prod,
                          _bap(apb[:, ic, :, :], [[3, nh], [0, 3], [1, 3]]),
                          _bap(frm_sb[:, ic, :], [[0, nh], [3, 3], [1, 3]]))
                      nc.vector.reduce_sum(
                          out=fp[:, ic, :].rearrange("p (x a) -> p x a", a=3),
                          in_=prod.rearrange("p h a b -> p (h a) b"),
                          axis=mybir.AxisListType.X)
                  po = 64 + hlo * 3
                  nc.vector.tensor_copy(fa1_a[:, :, po:po + nh * 3], fp)
                  fsq = ep.tile([128, IC, nh * 3], F32, tag="s36d", name="fsq")
                  nc.vector.tensor_mul(fsq, fp, fp)
                  nc.vector.reduce_sum(
                      out=dall[:, :, nc_:nc_ + nh],
                      in_=fsq.rearrange("p b (x a) -> p b x a", a=3),
                      axis=mybir.AxisListType.X)
                  defer[part] = fp

              def _emit_sqrt(c0, c1):
                  """sqrt + rsqrt of dall[:,:,c0:c1] via one ln + two exps."""
                  lnd = ep.tile([128, IC, c1 - c0], F32, tag="lnd", name="lnd")
                  nc.scalar.activation(out=lnd, in_=dall[:, :, c0:c1],
                                       func=AF.Ln, bias=tiny_sb, scale=1.0)
                  nc.scalar.activation(out=sqall[:, :, c0:c1], in_=lnd,
                                       func=AF.Exp, scale=0.5)
                  nc.scalar.activation(out=rsqall[:, :, c0:c1], in_=lnd,
                                       func=AF.Exp, scale=-0.5)

              def _emit_geo_tail(part):
                  hlo, hhi, dc, nc_ = _PARTS[part]
                  nh = hhi - hlo
                  fp = defer[part]
                  nc.vector.tensor_copy(fa1_a[:, :, 100 + hlo:100 + hhi],
                                        sqall[:, :, dc:dc + nh])
                  rn = rsqall[:, :, nc_:nc_ + 1]
                  dire = ep.tile([128, IC, nh * 3], F32, tag="s36e",
                                 name="dire")
                  nc.vector.tensor_mul(
                      dire.rearrange("p b (h a) -> p b h a", a=3),
                      fp.rearrange("p b (h a) -> p b h a", a=3),
                      _bap(rn, [[24, IC], [1, nh], [0, 3]]))
                  # dir cols 112+3*hlo .. 112+3*hhi, crossing into fa2 at 128
                  lo = 112 + 3 * hlo
                  hi = 112 + 3 * hhi
                  if lo < 128 and hi > 128:
                      nc.vector.tensor_copy(fa1_a[:, :, lo:128],
                                            dire[:, :, 0:128 - lo])
                      nc.vector.tensor_copy(fa2_a[:, :, 0:hi - 128],
                                            dire[:, :, 128 - lo:])
                  elif hi <= 128:
                      nc.vector.tensor_copy(fa1_a[:, :, lo:hi], dire)
                  else:
                      nc.vector.tensor_copy(fa2_a[:, :, lo - 128:hi - 128],
                                            dire)

              avt = {}

              def _emit_ft(g, hlf):
                  av = avt.pop((g, hlf))
                  co = (4 * g + 2 * hlf) * 20
                  nc.vector.tensor_copy(
                      ft_all[:, :, co:co + 40],
                      av[:].rearrange("p (b c) -> p b c", b=IC))

              # a "half" = one head x 512 queries x one key block: the unit
              # of both logits matmuls (N=512) and AV consumption
              halves = [(g, hlf, jb, t2)
                        for g in range(G) for hlf in range(2)
                        for jb in range(JB) for t2 in range(2)]
              # trivial: 3 halves per exp ([128,1536] = 3-bank tile, true
              # double buffering); general: 2 (uniform per-key-block bias)
              gw = 3 if trivial else 2
              groups = [halves[i:i + gw] for i in range(0, len(halves), gw)]

              backlog = []

              def _sqrt_a():
                  _emit_sqrt(0, 20)
                  _emit_geo_tail(0)
                  _emit_geo_tail("1a")
                  defer["sqA_done"] = True

              def _flush(idxs, e):
                  for q, (g, hlf, jb, t2) in enumerate(idxs):
                      first = False
                      if (g, hlf) not in avt:
                          avt[(g, hlf)] = psav.tile(
                              [128, IC * 40], F32, tag="av", name=f"av{g}{hlf}")
                          first = True
                      av = avt[(g, hlf)]
                      h = 4 * g + 2 * hlf + t2
                      for ic in range(IC):
                          nc.tensor.matmul(
                              av[:, ic * 40 + t2 * 20: ic * 40 + t2 * 20 + 20],
                              e[:, q * 512 + ic * 128: q * 512 + (ic + 1) * 128],
                              apk[:, jb, h, :],
                              start=(first and ic == 0),
                              stop=(jb == JB - 1),
                              skip_group_check=True)
                      if jb == JB - 1 and t2 == 1:
                          _emit_ft(g, hlf)
                          if (g, hlf) == (1, 1):
                              backlog.append(lambda: _emit_geo(0))
                              for _ic in range(IC):
                                  backlog.append(
                                      lambda ic=_ic: nc.sync.dma_start_transpose(
                                          out=fxt0_sb[:, ic, :],
                                          in_=fa0_a[:, ic, :]))
                          elif (g, hlf) == (2, 0):
                              backlog.append(lambda: _emit_geo("1a"))
                              backlog.append(_sqrt_a)

              pend = []
              for idxs in groups:
                  wide = len(idxs) * 512
                  lg = pslg.tile([128, gw * 512], F32, tag="lg", name="lg")
                  for q, (g, hlf, jb, t2) in enumerate(idxs):
                      t = 2 * hlf + t2
                      nc.tensor.matmul(
                          lg[:, q * 512:(q + 1) * 512],
                          ktp_sb[32 * t:32 * t + 16, g,
                                 jb * 128:(jb + 1) * 128],
                          qtp_sb[32 * t:32 * t + 16, g, :],
                          start=True, stop=True,
                          tile_position=(32 * t, 0))
                  e = epool.tile([128, wide], BF16, tag="E", name="e", bufs=3)
                  if trivial:
                      nc.scalar.activation(out=e, in_=lg[:, 0:wide],
                                           func=AF.Exp, scale=1.0)
                  else:
                      jb = idxs[0][2]
                      nc.scalar.activation(out=e, in_=lg[:, 0:wide],
                                           func=AF.Exp,
                                           bias=expb_sb[:, jb:jb + 1],
                                           scale=1.0)
                  # deferred geo emission: one batch per group, a group late,
                  # so its DVE inputs are long since ready and the strict-FIFO
                  # ACT (for the sqrt batch) never stalls the exp chain
                  if backlog:
                      backlog.pop(0)()
                  pend.append((idxs, e))
                  if len(pend) > 2:
                      _flush(*pend.pop(0))
              while pend:
                  _flush(*pend.pop(0))
              while backlog:
                  backlog.pop(0)()
              if "sqA_done" not in defer:
                  _sqrt_a()

              _emit_geo("1b")
              _emit_sqrt(20, 24)
              _emit_geo_tail("1b")

              # feat_all^T via transposes, then @ Wo ; residual + LN
              for ic in range(IC):
                  wo_ps = psav.tile([128, 128], F32, tag="av", name="wops")
                  fas = [(fa1_a[:, ic, :], 128), (fa2_a[:, ic, :], 32)]
                  tp = pslg.tile([128, 384], F16, tag="lg", name="tpa")
                  for cc, (fax, kk) in enumerate(fas, start=1):
                      nc.tensor.transpose(tp[0:kk, cc * 128:cc * 128 + 128],
                                          fax, identb)
                  fxt = ep.tile([128, 384], F16, tag="fxt", name="fxt")
                  nc.scalar.copy(fxt[:, 128:256], tp[:, 128:256])
                  nc.vector.tensor_copy(fxt[0:32, 256:384], tp[0:32, 256:384])
                  for cc, kk in enumerate((128, 128, 20)):
                      rhs = (wo0_sb, wo1_sb, wo2_sb)[cc]
                      lhsT = fxt0_sb[:, ic, :] if cc == 0 \
                          else fxt[0:kk, cc * 128:cc * 128 + 128]
                      nc.tensor.matmul(wo_ps[:, 0:128], lhsT, rhs,
                                       start=(cc == 0), stop=(cc == 2))
                  y = ep.tile([128, 128], F32, tag="y", name="y")
                  if trivial:
                      nc.vector.tensor_add(y, wo_ps[:, 0:128], xbo[:, ic, :])
                  else:
                      nc.vector.scalar_tensor_tensor(
                          out=y, in0=wo_ps[:, 0:128], scalar=msk[ic],
                          in1=xbo[:, ic, :], op0=ALU.mult, op1=ALU.add)
                  st6 = ep.tile([128, 6], F32, tag="st6", name="st6")
                  nc.vector.bn_stats(out=st6, in_=y)
                  mv = ep.tile([128, 2], F32, tag="mv", name="mv")
                  nc.vector.bn_aggr(out=mv, in_=st6)
                  # rstd = exp(-0.5 * ln(var + eps))
                  lnv = ep.tile([128, 1], F32, tag="lnv", name="lnv")
                  nc.scalar.activation(out=lnv, in_=mv[:, 1:2], func=AF.Ln,
                                       bias=eps_sb, scale=1.0)
                  rstd = ep.tile([128, 1], F32, tag="rstd", name="rstd")
                  nc.scalar.activation(out=rstd, in_=lnv, func=AF.Exp,
                                       scale=-0.5)
                  xc = ep.tile([128, 128], F32, tag="xc", name="xc")
                  nc.vector.tensor_scalar(out=xc, in0=y, scalar1=mv[:, 0:1],
                                          scalar2=rstd, op0=ALU.subtract,
                                          op1=ALU.mult)
                  if trivial:
                      o1 = xc
                  else:
                      o1 = ep.tile([128, 128], F32, tag="o1", name="o1")
                      nc.vector.tensor_mul(o1, xc, gmb_sb)
                      nc.vector.tensor_add(o1, o1, btb_sb)
                  eng = (nc.sync, nc.scalar, nc.sync, nc.scalar)[ic]
                  eng.dma_start(
                      out=out[:].rearrange("(c p) d -> c p d", p=128)[ic], in_=o1)

            for _rep in range(reps):
                _one_pass()

    # force the single ln+exp table set (ids preserved; see helper above)
    _bacc_mod.get_activation_tables = _only_ln_exp_tables
    try:
        nc.compile()
    finally:
        _bacc_mod.get_activation_tables = _orig_act_tables
    return nc


def _pm(a, nb):
    """[nb*128, F] -> partition-major [128, nb*F]."""
    f = a.shape[-1]
    return np.ascontiguousarray(
        a.reshape(nb, 128, f).transpose(1, 0, 2).reshape(128, nb * f))


def kernel(x, pos_CA, pos_CB, frame, mask, Wq, Wk, Wv, Wo, bo, gamma, beta):
    x = np.asarray(x, np.float32)
    pos_CA = np.asarray(pos_CA, np.float32)
    pos_CB = np.asarray(pos_CB, np.float32)
    frame = np.asarray(frame, np.float32)
    maskf = np.asarray(mask).astype(np.float32)
    Wq = np.asarray(Wq, np.float32)
    Wk = np.asarray(Wk, np.float32)
    Wv = np.asarray(Wv, np.float32)
    Wo = np.asarray(Wo, np.float32)
    bo = np.asarray(bo, np.float32)
    gamma = np.asarray(gamma, np.float32)
    beta = np.asarray(beta, np.float32)

    trivial = bool(
        maskf.all()
        and not bo.any()
        and (gamma == 1.0).all()
        and not beta.any()
    )
    key = ("nc", trivial)
    if key not in _compiled:
        _compiled[key] = _build(trivial=trivial)
        _compiled["nc"] = _compiled[key]
    nc = _compiled[key]
    _compiled["nc"] = nc

    wo01 = np.ascontiguousarray(np.vstack([Wo[0:256, :],]))
    wo2 = np.ascontiguousarray(Wo[256:276, :])
    bob = np.ascontiguousarray(np.tile(bo[None, :], (128, 1)))
    gmb = np.ascontiguousarray(np.tile(gamma[None, :], (128, 1)))
    btb = np.ascontiguousarray(np.tile(beta[None, :], (128, 1)))

    in_maps = []
    for c in range(NCORES):
        n, hf = c // 2, c % 2
        xn = x[n]
        sl = slice(hf * 512, (hf + 1) * 512)
        q = xn[sl] @ Wq                       # [512, 192]
        k = xn @ Wk                           # [1024, 192]
        v = xn @ Wv                           # [1024, 192]
        qtp_h = np.zeros((128, G, 512), np.float16)
        ktp_h = np.zeros((128, G, 1024), np.float16)
        for g in range(G):
            for t in range(4):
                h = 4 * g + t
                qtp_h[32 * t:32 * t + 16, g, :] = q[:, h * 16:(h + 1) * 16].T
                ktp_h[32 * t:32 * t + 16, g, :] = k[:, h * 16:(h + 1) * 16].T
        apk_h = np.ones((128, JB, H, 20), ml_dtypes.bfloat16)
        vr = v.reshape(JB, 128, H, 16).transpose(1, 0, 2, 3)
        apk_h[:, :, :, 0:16] = vr.astype(ml_dtypes.bfloat16)
        apk_h[:, :, :, 16:19] = pos_CB[n].reshape(JB, 128, 1, 3).transpose(
            1, 0, 2, 3).astype(ml_dtypes.bfloat16)
        in_maps.append({
            "qtp": qtp_h.reshape(128, G * 512),
            "ktp": ktp_h.reshape(128, G * 1024),
            "apkh": np.ascontiguousarray(apk_h.reshape(128, JB * H * 20)),
            "xq": _pm(xn[sl], 4),
            "pca": _pm(pos_CA[n, sl], 4),
            "frm": _pm(frame[n, sl].reshape(512, 9), 4),
            "expb": np.ascontiguousarray(
                (-INF * (1.0 - maskf[n])).reshape(8, 128).T),
            "mski": np.ascontiguousarray(maskf[n, sl].reshape(4, 128).T),
            "wo01": wo01.astype(np.float16),
            "wo2": wo2.astype(np.float16),
            "bob": bob, "gmb": gmb, "btb": btb,
        })

    res = bass_utils.run_bass_kernel_spmd(nc, in_maps, core_ids=list(range(NCORES)))
    full = np.empty((N, L, D), np.float32)
    for c in range(NCORES):
        n, hf = c // 2, c % 2
        full[n, hf * 512:(hf + 1) * 512, :] = res.results[c]["out"]
    return full

